# revision 26
# baseline (speedup 1.0000x reference)
"""Multi-head latent attention (MLA-style) Trainium2 kernel, 8-core SPMD.

Sharding: tensor-parallel over (batch x heads). Core c handles batch
b = c // 4 and the 4 heads 4*(c%4) .. 4*(c%4)+3:
  - kv latent (Wdkv) computed per core for its batch only
  - per-head compressed q, latent-space causal attention, and the head's
    slice of the output projection (row-sharded out_w)
  - per-core output is a PARTIAL [T, C] sum for its batch; host adds the
    4 partials per batch and the output bias.

All matmuls run in bf16 (fp32 PSUM accumulation).

Layouts (host-prepared):
  xT     [8, 128, T]      x[b].T              (c = o*128 + p)
  lw     [8, 128, 289]    latent_w, zero-padded col 288
  lbt    [128, 3]         latent_b per l-tile (fp32)
  wd     [8, 128, 1152]   Wd_w[h]/8 for the core's 4 heads, h*288+l
  wd2    [8, 128, 128]    Wd_w[h][:, 256:288]/8 stacked over 4 heads
  wdbt   [128, 12]        Wd_b[h]/8 per (h, l-tile) (fp32)
  wdbt2  [128, 1]         Wd_b[h][256:288]/8 stacked (fp32)
  ow     [8, 128, 1024]   out_w rows per (h, lt in 0..1)
  ow2    [128, 1024]      out_w l2 rows stacked over 4 heads
  masks  [4, 128, 512]    causal masks for the 4 diagonal key tiles
Output:
  out_p  [2048, 1024] fp32 partial (for the core's batch)
"""

import numpy as np
import ml_dtypes

B, T, C = 2, 2048, 1024
H, L = 16, 288
NCORES = 8
HPC = 4  # heads per core
CPB = NCORES // B  # cores per batch

# l-dimension tiles of L=288 (and the +1 sum row for the y matmul)
LT = [(0, 128), (1, 128), (2, 32)]
MT = [(0, 128), (1, 128), (2, 33)]  # y-matmul M tiles (includes sum row 288)

_cache = {}


def _build_nc():
    import concourse.bacc as bacc
    import concourse.mybir as mybir
    import concourse.tile as tile
    from concourse.bass import ts

    bf16 = mybir.dt.bfloat16
    f32 = mybir.dt.float32

    nc = bacc.Bacc("TRN2", target_bir_lowering=False, debug=True)

    d_xT = nc.dram_tensor("xT", [8, 128, T], bf16, kind="ExternalInput")
    d_xTs = nc.dram_tensor("xTs", [8, 128, 512], bf16, kind="ExternalInput")
    d_lw = nc.dram_tensor("lw", [8, 128, 289], bf16, kind="ExternalInput")
    d_lbt = nc.dram_tensor("lbt", [128, 3], f32, kind="ExternalInput")
    d_wd = nc.dram_tensor("wd", [8, 128, 1152], bf16, kind="ExternalInput")
    d_wd2 = nc.dram_tensor("wd2", [8, 128, 128], bf16, kind="ExternalInput")
    d_wdbt = nc.dram_tensor("wdbt", [128, 12], f32, kind="ExternalInput")
    d_wdbt2 = nc.dram_tensor("wdbt2", [128, 1], f32, kind="ExternalInput")
    d_ow = nc.dram_tensor("ow", [8, 128, 1024], bf16, kind="ExternalInput")
    d_ow2 = nc.dram_tensor("ow2", [128, 1024], bf16, kind="ExternalInput")
    d_masks = nc.dram_tensor("masks", [4, 128, 512], bf16, kind="ExternalInput")
    d_id = nc.dram_tensor("id128", [128, 128], bf16, kind="ExternalInput")
    d_out = nc.dram_tensor("out_p", [T, C], f32, kind="ExternalOutput")

    Exp = mybir.ActivationFunctionType.Exp
    Ident = mybir.ActivationFunctionType.Identity
    Ln = mybir.ActivationFunctionType.Ln

    with tile.TileContext(nc) as tc:
        with (
            tc.tile_pool(name="const", bufs=1) as cpool,
            tc.tile_pool(name="xp", bufs=1) as xpool,
            tc.tile_pool(name="kvp", bufs=1) as kvpool,
            tc.tile_pool(name="qp", bufs=2) as qpool,
            tc.tile_pool(name="ep", bufs=4) as epool,
            tc.tile_pool(name="yp", bufs=2) as ypool,
            tc.tile_pool(name="rp", bufs=2) as rpool,
            tc.tile_pool(name="op", bufs=3) as opool,
            tc.tile_pool(name="ps_y", bufs=1, space="PSUM") as ps_y,
            tc.tile_pool(name="ps_s", bufs=3, space="PSUM") as ps_s,
            tc.tile_pool(name="ps_m", bufs=2, space="PSUM") as ps_m,
            tc.tile_pool(name="dram", bufs=1, space="DRAM") as dram,
        ):
            # ---- persistent weights ----
            # latent_w first: the kvT matmuls only need lw + the first x
            # chunk, so the PE can start ~10us earlier
            lw_sb = cpool.tile([128, 8, 289], bf16, name="lw_sb")
            for kc in range(8):
                # split across the two HWDGE queues to halve the startup
                # serial chain (kv matmuls consume kc in order)
                eng = nc.sync if kc % 2 == 0 else nc.scalar
                eng.dma_start(lw_sb[:, kc, :], d_lw[kc])
            # the core's kv-shard x-slice: shortest path to firing the
            # AllGather (SWDGE queue, parallel to the HWDGE weight loads)
            xts_s = xpool.tile([128, 8, 512], bf16, name="xts_s", tag="xTs")
            for o in range(8):
                nc.gpsimd.dma_start(xts_s[:, o, :], d_xTs[o])
            lbt_sb = cpool.tile([128, 3], f32, name="lbt_sb")
            nc.sync.dma_start(lbt_sb[:], d_lbt[:])
            id_sb = cpool.tile([128, 128], bf16, name="id_sb")
            nc.sync.dma_start(id_sb[:], d_id[:])
            wd_sb = cpool.tile([128, 8, 1152], bf16, name="wd_sb")
            wd2_sb = cpool.tile([128, 8, 128], bf16, name="wd2_sb")
            wdbt_sb = cpool.tile([128, 12], f32, name="wdbt_sb")
            wdbt2_sb = cpool.tile([128, 1], f32, name="wdbt2_sb")
            ow_sb = cpool.tile([128, 8, 1024], bf16, name="ow_sb")
            ow2_sb = cpool.tile([128, 1024], bf16, name="ow2_sb")
            masks_sb = cpool.tile([128, 4, 512], bf16, name="masks_sb")

            def load_weights():
                for kc in range(8):
                    eng = nc.sync if kc % 2 == 0 else nc.scalar
                    eng.dma_start(wd_sb[:, kc, :], d_wd[kc])
                    eng.dma_start(wd2_sb[:, kc, :], d_wd2[kc])
                nc.sync.dma_start(wdbt_sb[:], d_wdbt[:])
                nc.scalar.dma_start(wdbt2_sb[:], d_wdbt2[:])
                for i in range(8):
                    eng = nc.sync if i % 2 == 0 else nc.scalar
                    eng.dma_start(ow_sb[:, i, :], d_ow[i])
                nc.sync.dma_start(ow2_sb[:], d_ow2[:])
                for i in range(4):
                    eng = nc.sync if i % 2 == 0 else nc.scalar
                    eng.dma_start(masks_sb[:, i, :], d_masks[i])

            # deferred out-projection: (yts, qc) emitted one head late so
            # the PE queue never blocks on the normalize chain
            pending = []

            def emit_outproj():
                yts, yt2s, pqc = pending.pop()
                for blk in range(4):
                    osb = opool.tile([128, 1024], f32, name="osb", tag="osb")
                    for cc in range(2):
                        po = ps_m.tile([128, 512], f32, name="ps_o", tag="m")
                        for h in range(HPC):
                            for lt in (0, 1):
                                nc.tensor.matmul(
                                    po,
                                    yts[h][:, lt, ts(blk, 128)],
                                    ow_sb[:, h * 2 + lt, ts(cc, 512)],
                                    start=(h == 0 and lt == 0),
                                    stop=False,
                                )
                        # all 4 heads' l2 blocks stacked into one K=128 matmul
                        nc.tensor.matmul(
                            po,
                            yt2s[:, ts(blk, 128)],
                            ow2_sb[:, ts(cc, 512)],
                            start=False,
                            stop=True,
                        )
                        nc.vector.tensor_copy(osb[:, ts(cc, 512)], po[:])
                    row0 = pqc * 512 + blk * 128
                    nc.sync.dma_start(d_out[row0 : row0 + 128, :], osb[:])

            # the last chunk's out-projection is emitted in two stages
            # (heads 0..2 overlap the last head's attention) to shrink the
            # end-of-kernel tail
            def emit_final_front(yts):
                osbs = []
                for blk in range(4):
                    osb = opool.tile(
                        [128, 1024], f32, name="osbf", tag="osbf", bufs=4
                    )
                    for cc in range(2):
                        po = ps_m.tile([128, 512], f32, name="ps_o", tag="m")
                        for h in range(HPC - 1):
                            for lt in (0, 1):
                                nc.tensor.matmul(
                                    po,
                                    yts[h][:, lt, ts(blk, 128)],
                                    ow_sb[:, h * 2 + lt, ts(cc, 512)],
                                    start=(h == 0 and lt == 0),
                                    stop=(h == HPC - 2 and lt == 1),
                                )
                        nc.vector.tensor_copy(osb[:, ts(cc, 512)], po[:])
                    osbs.append(osb)
                return osbs

            def emit_final_back(yt, yt2f, osbs, pqc):
                hl = HPC - 1
                for blk in range(4):
                    for cc in range(2):
                        po = ps_m.tile([128, 512], f32, name="ps_o", tag="m")
                        for lt in (0, 1):
                            nc.tensor.matmul(
                                po,
                                yt[:, lt, ts(blk, 128)],
                                ow_sb[:, hl * 2 + lt, ts(cc, 512)],
                                start=(lt == 0),
                                stop=False,
                            )
                        nc.tensor.matmul(
                            po,
                            yt2f[:, ts(blk, 128)],
                            ow2_sb[:, ts(cc, 512)],
                            start=False,
                            stop=True,
                        )
                        nc.vector.tensor_add(
                            osbs[blk][:, ts(cc, 512)],
                            po[:],
                            osbs[blk][:, ts(cc, 512)],
                        )
                    row0 = pqc * 512 + blk * 128
                    nc.sync.dma_start(d_out[row0 : row0 + 128, :], osbs[blk][:])

            # ---- load x^T, per 512-chunk (HWDGE queues; the SWDGE queue
            # stays clear for the collective + its loads) ----
            xts = []
            for tch in range(4):
                xt = xpool.tile([128, 8, 512], bf16, name="xt", tag=f"xT{tch}")
                for o in range(8):
                    eng = nc.sync if o % 2 == 0 else nc.scalar
                    eng.dma_start(xt[:, o, :], d_xT[o][:, ts(tch, 512)])
                xts.append(xt)
            load_weights()

            # ---- kvT = (x @ latent_w + latent_b)^T : [l, t], per chunk;
            #      kv_aug[t, 0:289] = [kv | 1] via PE transpose ----
            def compute_kv(xtile, ktag):
                kvt = kvpool.tile([128, 3, 512], bf16, name="kvt", tag=f"kvT{ktag}")
                for lt, lsz in LT:
                    pq = ps_s.tile([128, 512], f32, name="ps_kv", tag="s")
                    for kc in range(8):
                        nc.tensor.matmul(
                            pq[:lsz],
                            lw_sb[:, kc, lt * 128 : lt * 128 + lsz],
                            xtile[:, kc, :],
                            start=(kc == 0),
                            stop=(kc == 7),
                        )
                    nc.scalar.activation(
                        kvt[:lsz, lt, :],
                        pq[:lsz],
                        Ident,
                        bias=lbt_sb[:lsz, lt : lt + 1],
                    )

                # kv-l2 relaid out so adjacent t-tiles sit at partition
                # offsets 0/32, enabling paired (concurrent) K=32 matmuls
                kv2p = kvpool.tile([64, 2, 128], bf16, name="kv2p", tag=f"kv2p{ktag}")
                for j in range(4):
                    nc.sync.dma_start(
                        kv2p[32 * (j % 2) : 32 * (j % 2) + 32, j // 2, :],
                        kvt[:32, 2, ts(j, 128)],
                    )

                kva = kvpool.tile([128, 4, 289], bf16, name="kva", tag=f"kva{ktag}")
                for tt in range(4):
                    nc.vector.memset(kva[:, tt, 288:289], 1.0)
                    for lt, lsz in LT:
                        pt = ps_m.tile([128, 512], bf16, name="ps_t", tag="m")
                        nc.tensor.transpose(
                            pt[:, :lsz],
                            kvt[:lsz, lt, ts(tt, 128)],
                            id_sb[:lsz, :lsz],
                        )
                        nc.vector.tensor_copy(
                            kva[:, tt, lt * 128 : lt * 128 + lsz], pt[:, :lsz]
                        )
                return kvt, kv2p, kva

            # the core's shard chunk (chunk index c%4, selected by the host
            # via xTs), AllGathered across the 4-core batch group; chunk 0
            # is also computed locally so qc=0 attention starts immediately
            W_KVT, W_KVA, W_K2P = 3 * 512, 4 * 289, 2 * 128
            W_PACK = W_KVT + W_KVA + W_K2P
            kvt_s, kv2p_s, kva_s = compute_kv(xts_s, "s")
            cc_in = dram.tile([128, W_PACK], bf16, name="cc_in")
            cc_out = dram.tile([4, 128, W_PACK], bf16, name="cc_out")
            nc.gpsimd.dma_start(cc_in[:, 0:W_KVT], kvt_s[:])
            nc.gpsimd.dma_start(cc_in[:, W_KVT : W_KVT + W_KVA], kva_s[:])
            nc.gpsimd.dma_start(cc_in[:64, W_KVT + W_KVA :], kv2p_s[:])
            nc.gpsimd.collective_compute(
                "AllGather",
                mybir.AluOpType.bypass,
                replica_groups=[[0, 1, 2, 3], [4, 5, 6, 7]],
                ins=[cc_in.opt()],
                outs=[cc_out.opt()],
            )

            k0 = compute_kv(xts[0], 0)
            kvts, kv2ps, kvas = [k0[0]], [k0[1]], [k0[2]]
            for g in (1, 2, 3):
                kvt = kvpool.tile([128, 3, 512], bf16, name="kvt", tag=f"kvT{g}")
                kv2p = kvpool.tile([64, 2, 128], bf16, name="kv2p", tag=f"kv2p{g}")
                kva = kvpool.tile([128, 4, 289], bf16, name="kva", tag=f"kva{g}")
                eng = (nc.sync, nc.scalar, nc.gpsimd)[g - 1]
                eng.dma_start(kvt[:], cc_out[g][:, 0:W_KVT])
                eng.dma_start(kva[:], cc_out[g][:, W_KVT : W_KVT + W_KVA])
                eng.dma_start(kv2p[:], cc_out[g][:64, W_KVT + W_KVA :])
                kvts.append(kvt)
                kv2ps.append(kv2p)
                kvas.append(kva)

            # ---- attention per (chunk, head) ----
            for qc in range(4):
                final = qc == 3
                yts = []
                yt2s = ypool.tile([128, 512], bf16, name="yt2s", tag="yt2")

                # all 4 heads' l2 (l=256..287) q-projection stacked into
                # one M=128 matmul group; each head's half is then
                # DMA-replicated at partition offsets 0/32 so the paired
                # scores matmul K ranges line up
                pq2 = ps_s.tile([128, 512], f32, name="ps_q2", tag="s")
                for kc in range(8):
                    nc.tensor.matmul(
                        pq2,
                        wd2_sb[:, kc, :],
                        xts[qc][:, kc, :],
                        start=(kc == 0),
                        stop=(kc == 7),
                    )
                qt2w = qpool.tile([128, 512], bf16, name="qt2w", tag="qt2w")
                nc.scalar.activation(
                    qt2w[:], pq2[:], Ident, bias=wdbt2_sb[:, 0:1]
                )
                # per-head pair-replica: qrep[0:32,h]=qrep[32:64,h]=q2_h
                qrep = qpool.tile([64, 4, 512], bf16, name="qrep", tag="qrep")
                for h in range(HPC):
                    nc.sync.dma_start(qrep[0:32, h, :], qt2w[32 * h : 32 * h + 32, :])
                    nc.gpsimd.dma_start(
                        qrep[32:64, h, :], qt2w[32 * h : 32 * h + 32, :]
                    )

                for h in range(HPC):
                    # q^T chunk [l, 512] (scale 1/8 folded into wd)
                    qt = qpool.tile([128, 2, 512], bf16, name="qt", tag="qt")
                    for lt in (0, 1):
                        pq = ps_s.tile([128, 512], f32, name="ps_q", tag="s")
                        for kc in range(8):
                            nc.tensor.matmul(
                                pq,
                                wd_sb[:, kc, h * 288 + lt * 128 :][:, :128],
                                xts[qc][:, kc, :],
                                start=(kc == 0),
                                stop=(kc == 7),
                            )
                        nc.scalar.activation(
                            qt[:, lt, :],
                            pq[:],
                            Ident,
                            bias=wdbt_sb[:, h * 3 + lt : h * 3 + lt + 1],
                        )

                    # scores^T -> exp -> (mask) -> y accumulation
                    py = [
                        ps_y.tile([128, 512], f32, name=f"ps_y{mt}", tag=f"y{mt}")
                        for mt, _ in MT
                    ]
                    ntk = qc * 4 + 4

                    def emit_y(tk, et, c0):
                        for mt, msz in MT:
                            nc.tensor.matmul(
                                py[mt][:msz, c0:],
                                kvas[tk // 4][:, tk % 4, mt * 128 :][:, :msz],
                                et[:, c0:],
                                start=(tk == 0),
                                stop=(tk == ntk - 1),
                            )

                    # scores/exp pipelined one pair ahead of the y matmuls
                    # so the PE queue never blocks on the ACT exp; the two
                    # K=32 l2 matmuls of each pair run in concurrent PE
                    # row groups (partition offsets 0 / 32)
                    pend = []
                    for pr in range(ntk // 2):
                        pair = []
                        for tk in (2 * pr, 2 * pr + 1):
                            # diagonal tiles: only columns >= c0 unmasked
                            c0 = max(0, (tk - qc * 4) * 128)
                            pss = ps_s.tile(
                                [128, 512], f32, name="ps_s", tag="s"
                            )
                            for lt in (0, 1):
                                nc.tensor.matmul(
                                    pss[:, c0:],
                                    kvts[tk // 4][:, lt, ts(tk % 4, 128)],
                                    qt[:, lt, c0:],
                                    start=(lt == 0),
                                    stop=False,
                                )
                            pair.append((tk, pss, c0))
                        for off, (tk, pss, c0) in zip((0, 32), pair):
                            nc.tensor.matmul(
                                pss[:, c0:],
                                kv2ps[tk // 4][
                                    off : off + 32, (tk % 4) // 2, :
                                ],
                                qrep[off : off + 32, h, c0:],
                                start=False,
                                stop=True,
                            )
                        for tk, pss, c0 in pair:
                            et = epool.tile(
                                [128, 512], bf16, name="et", tag="et"
                            )
                            nc.scalar.activation(et[:, c0:], pss[:, c0:], Exp)
                            i = tk - qc * 4
                            if i >= 0:
                                # mask is nontrivial only in the i-th
                                # 128-column block
                                nc.vector.tensor_mul(
                                    et[:, c0 : c0 + 128],
                                    et[:, c0 : c0 + 128],
                                    masks_sb[:, i, c0 : c0 + 128],
                                )
                            pend.append((tk, et, c0))
                        while len(pend) > 2:
                            emit_y(*pend.pop(0))
                        if final and h == HPC - 1 and pr == 3:
                            # earlier heads' deferred out-projection, emitted
                            # here so its matmuls enter the PE queue well
                            # after their normalize chains have completed
                            final_osbs = emit_final_front(yts)
                    for e in pend:
                        emit_y(*e)

                    # drain the PSUM banks immediately (unnormalized), so the
                    # next head's matmuls never wait on the normalize chain
                    lnw = rpool.tile([1, 512], f32, name="lnw", tag="lnw")
                    nc.scalar.activation(lnw[:], py[2][32:33, :], Ln)
                    yu = ypool.tile(
                        [128, 2, 512], bf16, name="yu", tag=f"yu{h}", bufs=1
                    )
                    for lt in (0, 1):
                        nc.vector.tensor_copy(yu[:, lt, :], py[lt][:])
                    yu2 = rpool.tile([32, 512], bf16, name="yu2", tag=f"yu2{h}")
                    nc.vector.tensor_copy(yu2[:], py[2][:32])

                    # prev-head out-projection enqueues (PE + DVE copies)
                    # ahead of the normalize tail in the engine FIFOs
                    if pending:
                        emit_outproj()

                    # r = exp(-ln(sum)) = 1/sum, entirely on the scalar
                    # engine: keeps the serial 3.3us DVE reciprocal out of
                    # the DVE FIFO that the yt muls (and thus the deferred
                    # out-projection) queue behind
                    r_sb = rpool.tile([1, 512], f32, name="r_sb", tag="r")
                    nc.scalar.activation(r_sb[:], lnw[:], Exp, scale=-1.0)
                    rb_sb = rpool.tile([128, 512], f32, name="rb_sb", tag="rb")
                    nc.gpsimd.partition_broadcast(rb_sb[:], r_sb[:1, :])
                    yt = ypool.tile([128, 2, 512], bf16, name="yt", tag=f"yt{h}")
                    for lt in (0, 1):
                        nc.vector.tensor_mul(yt[:, lt, :], yu[:, lt, :], rb_sb[:])
                    nc.vector.tensor_mul(
                        yt2s[h * 32 : (h + 1) * 32, :], yu2[:], rb_sb[:32]
                    )
                    yts.append(yt)

                    if final and h == HPC - 1:
                        emit_final_back(yt, yt2s, final_osbs, qc)
                if not final:
                    pending.append((yts, yt2s, qc))

    nc.finalize()
    return nc


def _get_nc():
    if "nc" not in _cache:
        _cache["nc"] = _build_nc()
    return _cache["nc"]


def _prep_inputs(x, latent_w, latent_b, Wd_w, Wd_b, out_w):
    """Host-side shard + layout prep. Returns list of 8 per-core input maps."""
    bf16 = ml_dtypes.bfloat16
    x = np.asarray(x, dtype=np.float32)
    latent_w = np.asarray(latent_w, dtype=np.float32)
    latent_b = np.asarray(latent_b, dtype=np.float32)
    Wd_w = np.asarray(Wd_w, dtype=np.float32)
    Wd_b = np.asarray(Wd_b, dtype=np.float32)
    out_w = np.asarray(out_w, dtype=np.float32)

    xT = np.ascontiguousarray(x.transpose(0, 2, 1)).reshape(B, 8, 128, T)
    xT = xT.astype(bf16)

    lw = np.zeros((C, 289), np.float32)
    lw[:, :288] = latent_w
    lw = lw.reshape(8, 128, 289).astype(bf16)

    lbt = np.zeros((128, 3), np.float32)
    for lt, lsz in LT:
        lbt[:lsz, lt] = latent_b[lt * 128 : lt * 128 + lsz]

    # causal masks for the 4 diagonal key tiles: mask[i][tk, tq] = tq >= i*128+tk
    tq = np.arange(512)[None, :]
    tk = np.arange(128)[:, None]
    masks = np.stack([(tq >= i * 128 + tk) for i in range(4)]).astype(np.float32)
    masks = masks.astype(bf16)
    id128 = np.eye(128, dtype=np.float32).astype(bf16)

    # per-head-group weights (shared by the two cores of each group)
    grp_maps = []
    for g in range(CPB):
        heads = [HPC * g + i for i in range(HPC)]
        wd = np.zeros((8, 128, 1152), np.float32)
        wd2 = np.zeros((8, 128, 128), np.float32)
        wdbt = np.zeros((128, 12), np.float32)
        wdbt2 = np.zeros((128, 1), np.float32)
        ow = np.zeros((8, 128, 1024), np.float32)
        ow2 = np.zeros((128, 1024), np.float32)
        for i, h in enumerate(heads):
            ow2[i * 32 : (i + 1) * 32, :] = out_w[h * 288 + 256 : h * 288 + 288, :]
            wd2[:, :, i * 32 : (i + 1) * 32] = (
                Wd_w[h][:, 256:288] / 8.0
            ).reshape(8, 128, 32)
            wdbt2[i * 32 : (i + 1) * 32, 0] = Wd_b[h][256:288] / 8.0
            wd[:, :, i * 288 : (i + 1) * 288] = (Wd_w[h] / 8.0).reshape(8, 128, 288)
            for lt, lsz in LT:
                wdbt[:lsz, i * 3 + lt] = Wd_b[h][lt * 128 : lt * 128 + lsz] / 8.0
                if lt < 2:
                    ow[i * 2 + lt, :lsz, :] = out_w[
                        h * 288 + lt * 128 : h * 288 + lt * 128 + lsz, :
                    ]
        grp_maps.append(
            {
                "wd": wd.astype(bf16),
                "wd2": wd2.astype(bf16),
                "wdbt": wdbt,
                "wdbt2": wdbt2,
                "ow": ow.astype(bf16),
                "ow2": ow2.astype(bf16),
            }
        )

    in_maps = []
    for c in range(NCORES):
        b, g = divmod(c, CPB)
        m = {
            "xT": xT[b],
            "xTs": np.ascontiguousarray(xT[b][:, :, g * 512 : (g + 1) * 512]),
            "lw": lw,
            "lbt": lbt,
            "masks": masks,
            "id128": id128,
        }
        m.update(grp_maps[g])
        in_maps.append(m)
    return in_maps


def _combine(results, out_b):
    out = np.zeros((B, T, C), np.float64)
    for c in range(NCORES):
        out[c // CPB] += results[c]["out_p"].astype(np.float64)
    out += np.asarray(out_b, dtype=np.float64)[None, None, :]
    return out.astype(np.float32)


def kernel(x, latent_w, latent_b, Wd_w, Wd_b, out_w, out_b, **kw):
    from concourse import bass_utils

    nc = _get_nc()
    in_maps = _prep_inputs(x, latent_w, latent_b, Wd_w, Wd_b, out_w)
    res = bass_utils.run_bass_kernel_spmd(nc, in_maps, core_ids=list(range(NCORES)))
    return _combine(res.results, out_b)


# revision 33
# speedup vs baseline: 1.2079x; 1.2079x over previous
"""Multi-head latent attention (MLA-style) Trainium2 kernel, 8-core SPMD.

Sharding: tensor-parallel over (batch x heads). Core c handles batch
b = c // 4 and the 4 heads 4*(c%4) .. 4*(c%4)+3:
  - kv latent (Wdkv) computed per core for its batch only
  - per-head compressed q, latent-space causal attention, and the head's
    slice of the output projection (row-sharded out_w)
  - per-core output is a PARTIAL [T, C] sum for its batch; host adds the
    4 partials per batch and the output bias.

All matmuls run in bf16 (fp32 PSUM accumulation).

Layouts (host-prepared):
  xT     [8, 128, T]      x[b].T              (c = o*128 + p)
  lw     [8, 128, 289]    latent_w, zero-padded col 288
  lbt    [128, 3]         latent_b per l-tile (fp32)
  wd     [8, 128, 1152]   Wd_w[h]/8 for the core's 4 heads, h*288+l
  wd2    [8, 128, 128]    Wd_w[h][:, 256:288]/8 stacked over 4 heads
  wdbt   [128, 12]        Wd_b[h]/8 per (h, l-tile) (fp32)
  wdbt2  [128, 1]         Wd_b[h][256:288]/8 stacked (fp32)
  ow     [8, 128, 1024]   out_w rows per (h, lt in 0..1)
  ow2    [128, 1024]      out_w l2 rows stacked over 4 heads
  masks  [4, 128, 512]    causal masks for the 4 diagonal key tiles
Output:
  out_p  [2048, 1024] fp32 partial (for the core's batch)
"""

import numpy as np
import ml_dtypes

B, T, C = 2, 2048, 1024
H, L = 16, 288
NCORES = 8
HPC = 4  # heads per core
CPB = NCORES // B  # cores per batch

# l-dimension tiles of L=288 (and the +1 sum row for the y matmul)
LT = [(0, 128), (1, 128), (2, 32)]
MT = [(0, 128), (1, 128), (2, 33)]  # y-matmul M tiles (includes sum row 288)

_cache = {}


def _build_nc():
    import concourse.bacc as bacc
    import concourse.mybir as mybir
    import concourse.tile as tile
    from concourse.bass import ts

    bf16 = mybir.dt.bfloat16
    f32 = mybir.dt.float32

    nc = bacc.Bacc("TRN2", target_bir_lowering=False, debug=True)

    d_xT = nc.dram_tensor("xT", [8, 128, T], bf16, kind="ExternalInput")
    d_lw = nc.dram_tensor("lw", [8, 128, 289], bf16, kind="ExternalInput")
    d_lbt = nc.dram_tensor("lbt", [128, 3], f32, kind="ExternalInput")
    d_wd = nc.dram_tensor("wd", [8, 128, 1152], bf16, kind="ExternalInput")
    d_wd2 = nc.dram_tensor("wd2", [8, 128, 128], bf16, kind="ExternalInput")
    d_wdbt = nc.dram_tensor("wdbt", [128, 12], f32, kind="ExternalInput")
    d_wdbt2 = nc.dram_tensor("wdbt2", [128, 1], f32, kind="ExternalInput")
    d_ow = nc.dram_tensor("ow", [8, 128, 1024], bf16, kind="ExternalInput")
    d_ow2 = nc.dram_tensor("ow2", [128, 1024], bf16, kind="ExternalInput")
    d_masks = nc.dram_tensor("masks", [4, 128, 512], bf16, kind="ExternalInput")
    d_id = nc.dram_tensor("id128", [128, 128], bf16, kind="ExternalInput")
    d_out = nc.dram_tensor("out_p", [T, C], f32, kind="ExternalOutput")

    Exp = mybir.ActivationFunctionType.Exp
    Ident = mybir.ActivationFunctionType.Identity
    Ln = mybir.ActivationFunctionType.Ln

    with tile.TileContext(nc) as tc:
        with (
            tc.tile_pool(name="const", bufs=1) as cpool,
            tc.tile_pool(name="xp", bufs=1) as xpool,
            tc.tile_pool(name="kvp", bufs=1) as kvpool,
            tc.tile_pool(name="qp", bufs=2) as qpool,
            tc.tile_pool(name="ep", bufs=4) as epool,
            tc.tile_pool(name="yp", bufs=2) as ypool,
            tc.tile_pool(name="rp", bufs=2) as rpool,
            tc.tile_pool(name="op", bufs=3) as opool,
            tc.tile_pool(name="ps_y", bufs=1, space="PSUM") as ps_y,
            tc.tile_pool(name="ps_s", bufs=3, space="PSUM") as ps_s,
            tc.tile_pool(name="ps_m", bufs=2, space="PSUM") as ps_m,
        ):
            # ---- persistent weights ----
            # latent_w first: the kvT matmuls only need lw + the first x
            # chunk, so the PE can start ~10us earlier
            lw_sb = cpool.tile([128, 8, 289], bf16, name="lw_sb")
            for kc in range(8):
                # split across the two HWDGE queues to halve the startup
                # serial chain (kv matmuls consume kc in order)
                eng = nc.sync if kc % 2 == 0 else nc.scalar
                eng.dma_start(lw_sb[:, kc, :], d_lw[kc])

            lbt_sb = cpool.tile([128, 3], f32, name="lbt_sb")
            nc.sync.dma_start(lbt_sb[:], d_lbt[:])
            id_sb = cpool.tile([128, 128], bf16, name="id_sb")
            nc.sync.dma_start(id_sb[:], d_id[:])
            wd_sb = cpool.tile([128, 8, 1152], bf16, name="wd_sb")
            wd2_sb = cpool.tile([128, 8, 128], bf16, name="wd2_sb")
            wdbt_sb = cpool.tile([128, 12], f32, name="wdbt_sb")
            wdbt2_sb = cpool.tile([128, 1], f32, name="wdbt2_sb")
            ow_sb = cpool.tile([128, 8, 1024], bf16, name="ow_sb")
            ow2_sb = cpool.tile([128, 1024], bf16, name="ow2_sb")
            masks_sb = cpool.tile([128, 4, 512], bf16, name="masks_sb")

            def load_weights():
                for kc in range(8):
                    eng = nc.sync if kc % 2 == 0 else nc.scalar
                    eng.dma_start(wd_sb[:, kc, :], d_wd[kc])
                    eng.dma_start(wd2_sb[:, kc, :], d_wd2[kc])
                nc.sync.dma_start(wdbt_sb[:], d_wdbt[:])
                nc.scalar.dma_start(wdbt2_sb[:], d_wdbt2[:])
                for i in range(8):
                    eng = nc.sync if i % 2 == 0 else nc.scalar
                    eng.dma_start(ow_sb[:, i, :], d_ow[i])
                nc.sync.dma_start(ow2_sb[:], d_ow2[:])
                for i in range(4):
                    eng = nc.sync if i % 2 == 0 else nc.scalar
                    eng.dma_start(masks_sb[:, i, :], d_masks[i])

            # deferred out-projection: (yts, qc) emitted one head late so
            # the PE queue never blocks on the normalize chain
            pending = []

            def emit_outproj():
                yts, yt2s, pqc = pending.pop()
                for blk in range(4):
                    osb = opool.tile([128, 1024], f32, name="osb", tag="osb")
                    for cc in range(2):
                        po = ps_m.tile([128, 512], f32, name="ps_o", tag="m")
                        for h in range(HPC):
                            for lt in (0, 1):
                                nc.tensor.matmul(
                                    po,
                                    yts[h][:, lt, ts(blk, 128)],
                                    ow_sb[:, h * 2 + lt, ts(cc, 512)],
                                    start=(h == 0 and lt == 0),
                                    stop=False,
                                )
                        # all 4 heads' l2 blocks stacked into one K=128 matmul
                        nc.tensor.matmul(
                            po,
                            yt2s[:, ts(blk, 128)],
                            ow2_sb[:, ts(cc, 512)],
                            start=False,
                            stop=True,
                        )
                        nc.vector.tensor_copy(osb[:, ts(cc, 512)], po[:])
                    row0 = pqc * 512 + blk * 128
                    nc.sync.dma_start(d_out[row0 : row0 + 128, :], osb[:])

            # the last chunk's out-projection is emitted in two stages
            # (heads 0..2 overlap the last head's attention) to shrink the
            # end-of-kernel tail
            def emit_final_front(yts):
                osbs = []
                for blk in range(4):
                    osb = opool.tile(
                        [128, 1024], f32, name="osbf", tag="osbf", bufs=4
                    )
                    for cc in range(2):
                        po = ps_m.tile([128, 512], f32, name="ps_o", tag="m")
                        for h in range(HPC - 1):
                            for lt in (0, 1):
                                nc.tensor.matmul(
                                    po,
                                    yts[h][:, lt, ts(blk, 128)],
                                    ow_sb[:, h * 2 + lt, ts(cc, 512)],
                                    start=(h == 0 and lt == 0),
                                    stop=(h == HPC - 2 and lt == 1),
                                )
                        nc.vector.tensor_copy(osb[:, ts(cc, 512)], po[:])
                    osbs.append(osb)
                return osbs

            def emit_final_back(yt, yt2f, osbs, pqc):
                hl = HPC - 1
                for blk in range(4):
                    for cc in range(2):
                        po = ps_m.tile([128, 512], f32, name="ps_o", tag="m")
                        for lt in (0, 1):
                            nc.tensor.matmul(
                                po,
                                yt[:, lt, ts(blk, 128)],
                                ow_sb[:, hl * 2 + lt, ts(cc, 512)],
                                start=(lt == 0),
                                stop=False,
                            )
                        nc.tensor.matmul(
                            po,
                            yt2f[:, ts(blk, 128)],
                            ow2_sb[:, ts(cc, 512)],
                            start=False,
                            stop=True,
                        )
                        nc.vector.tensor_add(
                            osbs[blk][:, ts(cc, 512)],
                            po[:],
                            osbs[blk][:, ts(cc, 512)],
                        )
                    row0 = pqc * 512 + blk * 128
                    nc.sync.dma_start(d_out[row0 : row0 + 128, :], osbs[blk][:])

            # ---- load x^T, per 512-chunk (SWDGE queues, parallel to the
            # HWDGE weight loads) ----
            xts = []
            for tch in range(4):
                xt = xpool.tile([128, 8, 512], bf16, name="xt", tag=f"xT{tch}")
                for o in range(8):
                    nc.gpsimd.dma_start(xt[:, o, :], d_xT[o][:, ts(tch, 512)])
                xts.append(xt)
            load_weights()

            # ---- kvT = (x @ latent_w + latent_b)^T : [l, t], per chunk;
            #      kv_aug[t, 0:289] = [kv | 1] via PE transpose ----
            def compute_kv(xtile, ktag):
                kvt = kvpool.tile([128, 3, 512], bf16, name="kvt", tag=f"kvT{ktag}")
                for lt, lsz in LT:
                    pq = ps_s.tile([128, 512], f32, name="ps_kv", tag="s")
                    for kc in range(8):
                        nc.tensor.matmul(
                            pq[:lsz],
                            lw_sb[:, kc, lt * 128 : lt * 128 + lsz],
                            xtile[:, kc, :],
                            start=(kc == 0),
                            stop=(kc == 7),
                        )
                    nc.scalar.activation(
                        kvt[:lsz, lt, :],
                        pq[:lsz],
                        Ident,
                        bias=lbt_sb[:lsz, lt : lt + 1],
                    )

                # kv-l2 relaid out so adjacent t-tiles sit at partition
                # offsets 0/32, enabling paired (concurrent) K=32 matmuls
                kv2p = kvpool.tile([64, 2, 128], bf16, name="kv2p", tag=f"kv2p{ktag}")
                for j in range(4):
                    nc.sync.dma_start(
                        kv2p[32 * (j % 2) : 32 * (j % 2) + 32, j // 2, :],
                        kvt[:32, 2, ts(j, 128)],
                    )

                kva = kvpool.tile([128, 4, 289], bf16, name="kva", tag=f"kva{ktag}")
                for tt in range(4):
                    nc.vector.memset(kva[:, tt, 288:289], 1.0)
                    for lt, lsz in LT:
                        pt = ps_m.tile([128, 512], bf16, name="ps_t", tag="m")
                        nc.tensor.transpose(
                            pt[:, :lsz],
                            kvt[:lsz, lt, ts(tt, 128)],
                            id_sb[:lsz, :lsz],
                        )
                        nc.vector.tensor_copy(
                            kva[:, tt, lt * 128 : lt * 128 + lsz], pt[:, :lsz]
                        )
                return kvt, kv2p, kva

            kvts, kv2ps, kvas = [], [], []
            for tch in range(4):
                kvt, kv2p, kva = compute_kv(xts[tch], tch)
                kvts.append(kvt)
                kv2ps.append(kv2p)
                kvas.append(kva)

            # ---- attention per (chunk, head) ----
            for qc in range(4):
                final = qc == 3
                yts = []
                yt2s = ypool.tile([128, 512], bf16, name="yt2s", tag="yt2")

                # all 4 heads' l2 (l=256..287) q-projection stacked into
                # one M=128 matmul group; each head's half is then
                # DMA-replicated at partition offsets 0/32 so the paired
                # scores matmul K ranges line up
                pq2 = ps_s.tile([128, 512], f32, name="ps_q2", tag="s")
                for kc in range(8):
                    nc.tensor.matmul(
                        pq2,
                        wd2_sb[:, kc, :],
                        xts[qc][:, kc, :],
                        start=(kc == 0),
                        stop=(kc == 7),
                    )
                qt2w = qpool.tile([128, 512], bf16, name="qt2w", tag="qt2w")
                nc.scalar.activation(
                    qt2w[:], pq2[:], Ident, bias=wdbt2_sb[:, 0:1]
                )
                # per-head pair-replica: qrep[0:32,h]=qrep[32:64,h]=q2_h
                qrep = qpool.tile([64, 4, 512], bf16, name="qrep", tag="qrep")
                for h in range(HPC):
                    nc.sync.dma_start(qrep[0:32, h, :], qt2w[32 * h : 32 * h + 32, :])
                    nc.gpsimd.dma_start(
                        qrep[32:64, h, :], qt2w[32 * h : 32 * h + 32, :]
                    )

                # all heads' q^T hoisted to the chunk start so each head's
                # qt bias-activation hides under the next head's q matmuls
                # instead of stalling that head's first scores matmul
                # (scale 1/8 folded into wd)
                qts = []
                for h in range(HPC):
                    qt = qpool.tile(
                        [128, 2, 512], bf16, name="qt", tag=f"qt{h}", bufs=1
                    )
                    for lt in (0, 1):
                        pq = ps_s.tile([128, 512], f32, name="ps_q", tag="s")
                        for kc in range(8):
                            nc.tensor.matmul(
                                pq,
                                wd_sb[:, kc, h * 288 + lt * 128 :][:, :128],
                                xts[qc][:, kc, :],
                                start=(kc == 0),
                                stop=(kc == 7),
                            )
                        nc.scalar.activation(
                            qt[:, lt, :],
                            pq[:],
                            Ident,
                            bias=wdbt_sb[:, h * 3 + lt : h * 3 + lt + 1],
                        )
                    qts.append(qt)

                for h in range(HPC):
                    qt = qts[h]
                    # scores^T -> exp -> (mask) -> y accumulation
                    py = [
                        ps_y.tile([128, 512], f32, name=f"ps_y{mt}", tag=f"y{mt}")
                        for mt, _ in MT
                    ]
                    ntk = qc * 4 + 4

                    def emit_y(tk, et, c0):
                        for mt, msz in MT:
                            nc.tensor.matmul(
                                py[mt][:msz, c0:],
                                kvas[tk // 4][:, tk % 4, mt * 128 :][:, :msz],
                                et[:, c0:],
                                start=(tk == 0),
                                stop=(tk == ntk - 1),
                            )

                    # scores/exp pipelined one pair ahead of the y matmuls
                    # so the PE queue never blocks on the ACT exp; the two
                    # K=32 l2 matmuls of each pair run in concurrent PE
                    # row groups (partition offsets 0 / 32)
                    pend = []
                    for pr in range(ntk // 2):
                        pair = []
                        for tk in (2 * pr, 2 * pr + 1):
                            # diagonal tiles: only columns >= c0 unmasked
                            c0 = max(0, (tk - qc * 4) * 128)
                            pss = ps_s.tile(
                                [128, 512], f32, name="ps_s", tag="s"
                            )
                            for lt in (0, 1):
                                nc.tensor.matmul(
                                    pss[:, c0:],
                                    kvts[tk // 4][:, lt, ts(tk % 4, 128)],
                                    qt[:, lt, c0:],
                                    start=(lt == 0),
                                    stop=False,
                                )
                            pair.append((tk, pss, c0))
                        for off, (tk, pss, c0) in zip((0, 32), pair):
                            nc.tensor.matmul(
                                pss[:, c0:],
                                kv2ps[tk // 4][
                                    off : off + 32, (tk % 4) // 2, :
                                ],
                                qrep[off : off + 32, h, c0:],
                                start=False,
                                stop=True,
                            )
                        for tk, pss, c0 in pair:
                            et = epool.tile(
                                [128, 512], bf16, name="et", tag="et"
                            )
                            nc.scalar.activation(et[:, c0:], pss[:, c0:], Exp)
                            i = tk - qc * 4
                            if i >= 0:
                                # mask is nontrivial only in the i-th
                                # 128-column block
                                nc.vector.tensor_mul(
                                    et[:, c0 : c0 + 128],
                                    et[:, c0 : c0 + 128],
                                    masks_sb[:, i, c0 : c0 + 128],
                                )
                            pend.append((tk, et, c0))
                        while len(pend) > 2:
                            emit_y(*pend.pop(0))
                        if final and h == HPC - 1 and pr == 3:
                            # earlier heads' deferred out-projection, emitted
                            # here so its matmuls enter the PE queue well
                            # after their normalize chains have completed
                            final_osbs = emit_final_front(yts)
                    for e in pend:
                        emit_y(*e)

                    # drain the PSUM banks immediately (unnormalized), so the
                    # next head's matmuls never wait on the normalize chain
                    lnw = rpool.tile([1, 512], f32, name="lnw", tag="lnw")
                    nc.scalar.activation(lnw[:], py[2][32:33, :], Ln)
                    yu = ypool.tile(
                        [128, 2, 512], bf16, name="yu", tag=f"yu{h}", bufs=1
                    )
                    for lt in (0, 1):
                        nc.vector.tensor_copy(yu[:, lt, :], py[lt][:])
                    yu2 = rpool.tile([32, 512], bf16, name="yu2", tag=f"yu2{h}")
                    nc.vector.tensor_copy(yu2[:], py[2][:32])

                    # prev-head out-projection enqueues (PE + DVE copies)
                    # ahead of the normalize tail in the engine FIFOs
                    if pending:
                        emit_outproj()

                    # r = exp(-ln(sum)) = 1/sum, entirely on the scalar
                    # engine: keeps the serial 3.3us DVE reciprocal out of
                    # the DVE FIFO that the yt muls (and thus the deferred
                    # out-projection) queue behind
                    r_sb = rpool.tile([1, 512], f32, name="r_sb", tag="r")
                    nc.scalar.activation(r_sb[:], lnw[:], Exp, scale=-1.0)
                    rb_sb = rpool.tile([128, 512], f32, name="rb_sb", tag="rb")
                    nc.gpsimd.partition_broadcast(rb_sb[:], r_sb[:1, :])
                    yt = ypool.tile([128, 2, 512], bf16, name="yt", tag=f"yt{h}")
                    for lt in (0, 1):
                        nc.vector.tensor_mul(yt[:, lt, :], yu[:, lt, :], rb_sb[:])
                    nc.vector.tensor_mul(
                        yt2s[h * 32 : (h + 1) * 32, :], yu2[:], rb_sb[:32]
                    )
                    yts.append(yt)

                    if final and h == HPC - 1:
                        emit_final_back(yt, yt2s, final_osbs, qc)
                if not final:
                    pending.append((yts, yt2s, qc))

    nc.finalize()
    return nc


def _get_nc():
    if "nc" not in _cache:
        _cache["nc"] = _build_nc()
    return _cache["nc"]


def _prep_inputs(x, latent_w, latent_b, Wd_w, Wd_b, out_w):
    """Host-side shard + layout prep. Returns list of 8 per-core input maps."""
    bf16 = ml_dtypes.bfloat16
    x = np.asarray(x, dtype=np.float32)
    latent_w = np.asarray(latent_w, dtype=np.float32)
    latent_b = np.asarray(latent_b, dtype=np.float32)
    Wd_w = np.asarray(Wd_w, dtype=np.float32)
    Wd_b = np.asarray(Wd_b, dtype=np.float32)
    out_w = np.asarray(out_w, dtype=np.float32)

    xT = np.ascontiguousarray(x.transpose(0, 2, 1)).reshape(B, 8, 128, T)
    xT = xT.astype(bf16)

    lw = np.zeros((C, 289), np.float32)
    lw[:, :288] = latent_w
    lw = lw.reshape(8, 128, 289).astype(bf16)

    lbt = np.zeros((128, 3), np.float32)
    for lt, lsz in LT:
        lbt[:lsz, lt] = latent_b[lt * 128 : lt * 128 + lsz]

    # causal masks for the 4 diagonal key tiles: mask[i][tk, tq] = tq >= i*128+tk
    tq = np.arange(512)[None, :]
    tk = np.arange(128)[:, None]
    masks = np.stack([(tq >= i * 128 + tk) for i in range(4)]).astype(np.float32)
    masks = masks.astype(bf16)
    id128 = np.eye(128, dtype=np.float32).astype(bf16)

    # per-head-group weights (shared by the two cores of each group)
    grp_maps = []
    for g in range(CPB):
        heads = [HPC * g + i for i in range(HPC)]
        wd = np.zeros((8, 128, 1152), np.float32)
        wd2 = np.zeros((8, 128, 128), np.float32)
        wdbt = np.zeros((128, 12), np.float32)
        wdbt2 = np.zeros((128, 1), np.float32)
        ow = np.zeros((8, 128, 1024), np.float32)
        ow2 = np.zeros((128, 1024), np.float32)
        for i, h in enumerate(heads):
            ow2[i * 32 : (i + 1) * 32, :] = out_w[h * 288 + 256 : h * 288 + 288, :]
            wd2[:, :, i * 32 : (i + 1) * 32] = (
                Wd_w[h][:, 256:288] / 8.0
            ).reshape(8, 128, 32)
            wdbt2[i * 32 : (i + 1) * 32, 0] = Wd_b[h][256:288] / 8.0
            wd[:, :, i * 288 : (i + 1) * 288] = (Wd_w[h] / 8.0).reshape(8, 128, 288)
            for lt, lsz in LT:
                wdbt[:lsz, i * 3 + lt] = Wd_b[h][lt * 128 : lt * 128 + lsz] / 8.0
                if lt < 2:
                    ow[i * 2 + lt, :lsz, :] = out_w[
                        h * 288 + lt * 128 : h * 288 + lt * 128 + lsz, :
                    ]
        grp_maps.append(
            {
                "wd": wd.astype(bf16),
                "wd2": wd2.astype(bf16),
                "wdbt": wdbt,
                "wdbt2": wdbt2,
                "ow": ow.astype(bf16),
                "ow2": ow2.astype(bf16),
            }
        )

    in_maps = []
    for c in range(NCORES):
        b, g = divmod(c, CPB)
        m = {
            "xT": xT[b],
            "lw": lw,
            "lbt": lbt,
            "masks": masks,
            "id128": id128,
        }
        m.update(grp_maps[g])
        in_maps.append(m)
    return in_maps


def _combine(results, out_b):
    out = np.zeros((B, T, C), np.float64)
    for c in range(NCORES):
        out[c // CPB] += results[c]["out_p"].astype(np.float64)
    out += np.asarray(out_b, dtype=np.float64)[None, None, :]
    return out.astype(np.float32)


def kernel(x, latent_w, latent_b, Wd_w, Wd_b, out_w, out_b, **kw):
    from concourse import bass_utils

    nc = _get_nc()
    in_maps = _prep_inputs(x, latent_w, latent_b, Wd_w, Wd_b, out_w)
    res = bass_utils.run_bass_kernel_spmd(nc, in_maps, core_ids=list(range(NCORES)))
    return _combine(res.results, out_b)


# revision 36
# speedup vs baseline: 1.2079x; 1.0000x over previous
"""Multi-head latent attention (MLA-style) Trainium2 kernel, 8-core SPMD.

Sharding: tensor-parallel over (batch x heads). Core c handles batch
b = c // 4 and the 4 heads 4*(c%4) .. 4*(c%4)+3:
  - kv latent (Wdkv) computed per core for its batch only
  - per-head compressed q, latent-space causal attention, and the head's
    slice of the output projection (row-sharded out_w)
  - per-core output is a PARTIAL [T, C] sum for its batch; host adds the
    4 partials per batch and the output bias.

All matmuls run in bf16 (fp32 PSUM accumulation).

Layouts (host-prepared):
  xT     [8, 128, T]      x[b].T              (c = o*128 + p)
  lw     [8, 128, 289]    latent_w, zero-padded col 288
  lbt    [128, 3]         latent_b per l-tile (fp32)
  wd     [8, 128, 1152]   Wd_w[h]/8 for the core's 4 heads, h*288+l
  wd2    [8, 128, 128]    Wd_w[h][:, 256:288]/8 stacked over 4 heads
  wdbt   [128, 12]        Wd_b[h]/8 per (h, l-tile) (fp32)
  wdbt2  [128, 1]         Wd_b[h][256:288]/8 stacked (fp32)
  ow     [8, 128, 1024]   out_w rows per (h, lt in 0..1)
  ow2    [128, 1024]      out_w l2 rows stacked over 4 heads
  masks  [4, 128, 512]    causal masks for the 4 diagonal key tiles
Output:
  out_p  [2048, 1024] fp32 partial (for the core's batch)
"""

import numpy as np
import ml_dtypes

B, T, C = 2, 2048, 1024
H, L = 16, 288
NCORES = 8
HPC = 4  # heads per core
CPB = NCORES // B  # cores per batch

# l-dimension tiles of L=288 (and the +1 sum row for the y matmul)
LT = [(0, 128), (1, 128), (2, 32)]
MT = [(0, 128), (1, 128), (2, 33)]  # y-matmul M tiles (includes sum row 288)

_cache = {}


def _build_nc():
    import concourse.bacc as bacc
    import concourse.mybir as mybir
    import concourse.tile as tile
    from concourse.bass import ts

    bf16 = mybir.dt.bfloat16
    f32 = mybir.dt.float32

    nc = bacc.Bacc("TRN2", target_bir_lowering=False, debug=True)

    d_xT = nc.dram_tensor("xT", [8, 128, T], bf16, kind="ExternalInput")
    d_lw = nc.dram_tensor("lw", [8, 128, 289], bf16, kind="ExternalInput")
    d_lbt = nc.dram_tensor("lbt", [128, 3], f32, kind="ExternalInput")
    d_wd = nc.dram_tensor("wd", [8, 128, 1152], bf16, kind="ExternalInput")
    d_wd2 = nc.dram_tensor("wd2", [8, 128, 128], bf16, kind="ExternalInput")
    d_wdbt = nc.dram_tensor("wdbt", [128, 12], f32, kind="ExternalInput")
    d_wdbt2 = nc.dram_tensor("wdbt2", [128, 1], f32, kind="ExternalInput")
    d_ow = nc.dram_tensor("ow", [8, 128, 1024], bf16, kind="ExternalInput")
    d_ow2 = nc.dram_tensor("ow2", [128, 1024], bf16, kind="ExternalInput")
    d_masks = nc.dram_tensor("masks", [4, 128, 512], bf16, kind="ExternalInput")
    d_id = nc.dram_tensor("id128", [128, 128], bf16, kind="ExternalInput")
    d_out = nc.dram_tensor("out_p", [T, C], f32, kind="ExternalOutput")

    Exp = mybir.ActivationFunctionType.Exp
    Ident = mybir.ActivationFunctionType.Identity
    Ln = mybir.ActivationFunctionType.Ln

    with tile.TileContext(nc) as tc:
        with (
            tc.tile_pool(name="const", bufs=1) as cpool,
            tc.tile_pool(name="xp", bufs=1) as xpool,
            tc.tile_pool(name="kvp", bufs=1) as kvpool,
            tc.tile_pool(name="qp", bufs=2) as qpool,
            tc.tile_pool(name="ep", bufs=4) as epool,
            tc.tile_pool(name="yp", bufs=2) as ypool,
            tc.tile_pool(name="rp", bufs=2) as rpool,
            tc.tile_pool(name="op", bufs=3) as opool,
            tc.tile_pool(name="ps_y", bufs=1, space="PSUM") as ps_y,
            tc.tile_pool(name="ps_s", bufs=3, space="PSUM") as ps_s,
            tc.tile_pool(name="ps_m", bufs=2, space="PSUM") as ps_m,
        ):
            # ---- persistent weights ----
            # latent_w first: the kvT matmuls only need lw + the first x
            # chunk, so the PE can start ~10us earlier
            lw_sb = cpool.tile([128, 8, 289], bf16, name="lw_sb")
            for kc in range(8):
                # split across the two HWDGE queues to halve the startup
                # serial chain (kv matmuls consume kc in order)
                eng = nc.sync if kc % 2 == 0 else nc.scalar
                eng.dma_start(lw_sb[:, kc, :], d_lw[kc])

            lbt_sb = cpool.tile([128, 3], f32, name="lbt_sb")
            nc.sync.dma_start(lbt_sb[:], d_lbt[:])
            id_sb = cpool.tile([128, 128], bf16, name="id_sb")
            nc.sync.dma_start(id_sb[:], d_id[:])
            wd_sb = cpool.tile([128, 8, 1152], bf16, name="wd_sb")
            wd2_sb = cpool.tile([128, 8, 128], bf16, name="wd2_sb")
            wdbt_sb = cpool.tile([128, 12], f32, name="wdbt_sb")
            wdbt2_sb = cpool.tile([128, 1], f32, name="wdbt2_sb")
            ow_sb = cpool.tile([128, 8, 1024], bf16, name="ow_sb")
            ow2_sb = cpool.tile([128, 1024], bf16, name="ow2_sb")
            masks_sb = cpool.tile([128, 4, 512], bf16, name="masks_sb")

            def load_weights():
                for kc in range(8):
                    eng = nc.sync if kc % 2 == 0 else nc.scalar
                    eng.dma_start(wd_sb[:, kc, :], d_wd[kc])
                    eng.dma_start(wd2_sb[:, kc, :], d_wd2[kc])
                nc.sync.dma_start(wdbt_sb[:], d_wdbt[:])
                nc.scalar.dma_start(wdbt2_sb[:], d_wdbt2[:])
                for i in range(8):
                    eng = nc.sync if i % 2 == 0 else nc.scalar
                    eng.dma_start(ow_sb[:, i, :], d_ow[i])
                nc.sync.dma_start(ow2_sb[:], d_ow2[:])
                for i in range(4):
                    eng = nc.sync if i % 2 == 0 else nc.scalar
                    eng.dma_start(masks_sb[:, i, :], d_masks[i])

            # deferred out-projection: (yts, qc) emitted one head late so
            # the PE queue never blocks on the normalize chain
            pending = []

            def emit_outproj():
                yts, yt2s, pqc = pending.pop()
                for blk in range(4):
                    osb = opool.tile([128, 1024], f32, name="osb", tag="osb")
                    for cc in range(2):
                        po = ps_m.tile([128, 512], f32, name="ps_o", tag="m")
                        for h in range(HPC):
                            for lt in (0, 1):
                                nc.tensor.matmul(
                                    po,
                                    yts[h][:, lt, ts(blk, 128)],
                                    ow_sb[:, h * 2 + lt, ts(cc, 512)],
                                    start=(h == 0 and lt == 0),
                                    stop=False,
                                )
                        # all 4 heads' l2 blocks stacked into one K=128 matmul
                        nc.tensor.matmul(
                            po,
                            yt2s[:, ts(blk, 128)],
                            ow2_sb[:, ts(cc, 512)],
                            start=False,
                            stop=True,
                        )
                        nc.vector.tensor_copy(osb[:, ts(cc, 512)], po[:])
                    row0 = pqc * 512 + blk * 128
                    nc.sync.dma_start(d_out[row0 : row0 + 128, :], osb[:])

            # the last chunk's out-projection is emitted in two stages
            # (heads 0..2 overlap the last head's attention) to shrink the
            # end-of-kernel tail
            def emit_final_front(yts):
                osbs = []
                for blk in range(4):
                    osb = opool.tile(
                        [128, 1024], f32, name="osbf", tag="osbf", bufs=4
                    )
                    for cc in range(2):
                        po = ps_m.tile([128, 512], f32, name="ps_o", tag="m")
                        for h in range(HPC - 1):
                            for lt in (0, 1):
                                nc.tensor.matmul(
                                    po,
                                    yts[h][:, lt, ts(blk, 128)],
                                    ow_sb[:, h * 2 + lt, ts(cc, 512)],
                                    start=(h == 0 and lt == 0),
                                    stop=(h == HPC - 2 and lt == 1),
                                )
                        nc.vector.tensor_copy(osb[:, ts(cc, 512)], po[:])
                    osbs.append(osb)
                return osbs

            def emit_final_back(yt, yt2f, osbs, pqc):
                hl = HPC - 1
                for blk in range(4):
                    for cc in range(2):
                        po = ps_m.tile([128, 512], f32, name="ps_o", tag="m")
                        for lt in (0, 1):
                            nc.tensor.matmul(
                                po,
                                yt[:, lt, ts(blk, 128)],
                                ow_sb[:, hl * 2 + lt, ts(cc, 512)],
                                start=(lt == 0),
                                stop=False,
                            )
                        nc.tensor.matmul(
                            po,
                            yt2f[:, ts(blk, 128)],
                            ow2_sb[:, ts(cc, 512)],
                            start=False,
                            stop=True,
                        )
                        nc.vector.tensor_add(
                            osbs[blk][:, ts(cc, 512)],
                            po[:],
                            osbs[blk][:, ts(cc, 512)],
                        )
                    row0 = pqc * 512 + blk * 128
                    nc.sync.dma_start(d_out[row0 : row0 + 128, :], osbs[blk][:])

            # ---- load x^T, per 512-chunk (SWDGE queues, parallel to the
            # HWDGE weight loads) ----
            xts = []
            for tch in range(4):
                xt = xpool.tile([128, 8, 512], bf16, name="xt", tag=f"xT{tch}")
                for o in range(8):
                    nc.gpsimd.dma_start(xt[:, o, :], d_xT[o][:, ts(tch, 512)])
                xts.append(xt)
            load_weights()

            # ---- kvT = (x @ latent_w + latent_b)^T : [l, t], per chunk;
            #      kv_aug[t, 0:289] = [kv | 1] via PE transpose ----
            def compute_kv(xtile, ktag):
                kvt = kvpool.tile([128, 3, 512], bf16, name="kvt", tag=f"kvT{ktag}")
                # kc outer / lt inner: each newly-arrived x slice feeds all
                # three l-tile accumulations, so the matmuls pipeline with
                # the serial per-slice x DMAs instead of stalling on them
                pqs = [
                    ps_s.tile([128, 512], f32, name=f"ps_kv{lt}", tag="s")
                    for lt, _ in LT
                ]
                for kc in range(8):
                    for lt, lsz in LT:
                        nc.tensor.matmul(
                            pqs[lt][:lsz],
                            lw_sb[:, kc, lt * 128 : lt * 128 + lsz],
                            xtile[:, kc, :],
                            start=(kc == 0),
                            stop=(kc == 7),
                        )
                for lt, lsz in LT:
                    nc.scalar.activation(
                        kvt[:lsz, lt, :],
                        pqs[lt][:lsz],
                        Ident,
                        bias=lbt_sb[:lsz, lt : lt + 1],
                    )

                # kv-l2 relaid out so adjacent t-tiles sit at partition
                # offsets 0/32, enabling paired (concurrent) K=32 matmuls
                kv2p = kvpool.tile([64, 2, 128], bf16, name="kv2p", tag=f"kv2p{ktag}")
                for j in range(4):
                    nc.sync.dma_start(
                        kv2p[32 * (j % 2) : 32 * (j % 2) + 32, j // 2, :],
                        kvt[:32, 2, ts(j, 128)],
                    )

                kva = kvpool.tile([128, 4, 289], bf16, name="kva", tag=f"kva{ktag}")
                for tt in range(4):
                    nc.vector.memset(kva[:, tt, 288:289], 1.0)
                    for lt, lsz in LT:
                        pt = ps_m.tile([128, 512], bf16, name="ps_t", tag="m")
                        nc.tensor.transpose(
                            pt[:, :lsz],
                            kvt[:lsz, lt, ts(tt, 128)],
                            id_sb[:lsz, :lsz],
                        )
                        # alternate the PSUM->SBUF copies between the two
                        # vector-capable engines: a single copy lane is
                        # slower than the PE transposes and stalls them via
                        # the 2-buffer ps_m recycling
                        eng = nc.vector if (tt + lt) % 2 == 0 else nc.scalar
                        if eng is nc.vector:
                            eng.tensor_copy(
                                kva[:, tt, lt * 128 : lt * 128 + lsz],
                                pt[:, :lsz],
                            )
                        else:
                            nc.scalar.activation(
                                kva[:, tt, lt * 128 : lt * 128 + lsz],
                                pt[:, :lsz],
                                Ident,
                            )
                return kvt, kv2p, kva

            kvts, kv2ps, kvas = [], [], []
            for tch in range(4):
                kvt, kv2p, kva = compute_kv(xts[tch], tch)
                kvts.append(kvt)
                kv2ps.append(kv2p)
                kvas.append(kva)

            # ---- attention per (chunk, head) ----
            for qc in range(4):
                final = qc == 3
                yts = []
                yt2s = ypool.tile([128, 512], bf16, name="yt2s", tag="yt2")

                # all 4 heads' l2 (l=256..287) q-projection stacked into
                # one M=128 matmul group; each head's half is then
                # DMA-replicated at partition offsets 0/32 so the paired
                # scores matmul K ranges line up
                pq2 = ps_s.tile([128, 512], f32, name="ps_q2", tag="s")
                for kc in range(8):
                    nc.tensor.matmul(
                        pq2,
                        wd2_sb[:, kc, :],
                        xts[qc][:, kc, :],
                        start=(kc == 0),
                        stop=(kc == 7),
                    )
                qt2w = qpool.tile([128, 512], bf16, name="qt2w", tag="qt2w")
                nc.scalar.activation(
                    qt2w[:], pq2[:], Ident, bias=wdbt2_sb[:, 0:1]
                )
                # per-head pair-replica: qrep[0:32,h]=qrep[32:64,h]=q2_h
                qrep = qpool.tile([64, 4, 512], bf16, name="qrep", tag="qrep")
                for h in range(HPC):
                    nc.sync.dma_start(qrep[0:32, h, :], qt2w[32 * h : 32 * h + 32, :])
                    nc.gpsimd.dma_start(
                        qrep[32:64, h, :], qt2w[32 * h : 32 * h + 32, :]
                    )

                for h in range(HPC):
                    # q^T chunk [l, 512] (scale 1/8 folded into wd)
                    qt = qpool.tile([128, 2, 512], bf16, name="qt", tag="qt")
                    for lt in (0, 1):
                        pq = ps_s.tile([128, 512], f32, name="ps_q", tag="s")
                        for kc in range(8):
                            nc.tensor.matmul(
                                pq,
                                wd_sb[:, kc, h * 288 + lt * 128 :][:, :128],
                                xts[qc][:, kc, :],
                                start=(kc == 0),
                                stop=(kc == 7),
                            )
                        nc.scalar.activation(
                            qt[:, lt, :],
                            pq[:],
                            Ident,
                            bias=wdbt_sb[:, h * 3 + lt : h * 3 + lt + 1],
                        )

                    # scores^T -> exp -> (mask) -> y accumulation
                    py = [
                        ps_y.tile([128, 512], f32, name=f"ps_y{mt}", tag=f"y{mt}")
                        for mt, _ in MT
                    ]
                    ntk = qc * 4 + 4

                    def emit_y(tk, et, c0):
                        for mt, msz in MT:
                            nc.tensor.matmul(
                                py[mt][:msz, c0:],
                                kvas[tk // 4][:, tk % 4, mt * 128 :][:, :msz],
                                et[:, c0:],
                                start=(tk == 0),
                                stop=(tk == ntk - 1),
                            )

                    # scores/exp pipelined one pair ahead of the y matmuls
                    # so the PE queue never blocks on the ACT exp; the two
                    # K=32 l2 matmuls of each pair run in concurrent PE
                    # row groups (partition offsets 0 / 32)
                    pend = []
                    for pr in range(ntk // 2):
                        pair = []
                        for tk in (2 * pr, 2 * pr + 1):
                            # diagonal tiles: only columns >= c0 unmasked
                            c0 = max(0, (tk - qc * 4) * 128)
                            pss = ps_s.tile(
                                [128, 512], f32, name="ps_s", tag="s"
                            )
                            for lt in (0, 1):
                                nc.tensor.matmul(
                                    pss[:, c0:],
                                    kvts[tk // 4][:, lt, ts(tk % 4, 128)],
                                    qt[:, lt, c0:],
                                    start=(lt == 0),
                                    stop=False,
                                )
                            pair.append((tk, pss, c0))
                        for off, (tk, pss, c0) in zip((0, 32), pair):
                            nc.tensor.matmul(
                                pss[:, c0:],
                                kv2ps[tk // 4][
                                    off : off + 32, (tk % 4) // 2, :
                                ],
                                qrep[off : off + 32, h, c0:],
                                start=False,
                                stop=True,
                            )
                        for tk, pss, c0 in pair:
                            et = epool.tile(
                                [128, 512], bf16, name="et", tag="et"
                            )
                            nc.scalar.activation(et[:, c0:], pss[:, c0:], Exp)
                            i = tk - qc * 4
                            if i >= 0:
                                # mask is nontrivial only in the i-th
                                # 128-column block
                                nc.vector.tensor_mul(
                                    et[:, c0 : c0 + 128],
                                    et[:, c0 : c0 + 128],
                                    masks_sb[:, i, c0 : c0 + 128],
                                )
                            pend.append((tk, et, c0))
                        while len(pend) > 2:
                            emit_y(*pend.pop(0))
                        if final and h == HPC - 1 and pr == 3:
                            # earlier heads' deferred out-projection, emitted
                            # here so its matmuls enter the PE queue well
                            # after their normalize chains have completed
                            final_osbs = emit_final_front(yts)
                    for e in pend:
                        emit_y(*e)

                    # drain the PSUM banks immediately (unnormalized), so the
                    # next head's matmuls never wait on the normalize chain
                    lnw = rpool.tile([1, 512], f32, name="lnw", tag="lnw")
                    nc.scalar.activation(lnw[:], py[2][32:33, :], Ln)
                    yu = ypool.tile(
                        [128, 2, 512], bf16, name="yu", tag=f"yu{h}", bufs=1
                    )
                    for lt in (0, 1):
                        nc.vector.tensor_copy(yu[:, lt, :], py[lt][:])
                    yu2 = rpool.tile([32, 512], bf16, name="yu2", tag=f"yu2{h}")
                    nc.vector.tensor_copy(yu2[:], py[2][:32])

                    # prev-head out-projection enqueues (PE + DVE copies)
                    # ahead of the normalize tail in the engine FIFOs
                    if pending:
                        emit_outproj()

                    # r = exp(-ln(sum)) = 1/sum, entirely on the scalar
                    # engine: keeps the serial 3.3us DVE reciprocal out of
                    # the DVE FIFO that the yt muls (and thus the deferred
                    # out-projection) queue behind
                    r_sb = rpool.tile([1, 512], f32, name="r_sb", tag="r")
                    nc.scalar.activation(r_sb[:], lnw[:], Exp, scale=-1.0)
                    rb_sb = rpool.tile([128, 512], f32, name="rb_sb", tag="rb")
                    nc.gpsimd.partition_broadcast(rb_sb[:], r_sb[:1, :])
                    yt = ypool.tile([128, 2, 512], bf16, name="yt", tag=f"yt{h}")
                    for lt in (0, 1):
                        nc.vector.tensor_mul(yt[:, lt, :], yu[:, lt, :], rb_sb[:])
                    nc.vector.tensor_mul(
                        yt2s[h * 32 : (h + 1) * 32, :], yu2[:], rb_sb[:32]
                    )
                    yts.append(yt)

                    if final and h == HPC - 1:
                        emit_final_back(yt, yt2s, final_osbs, qc)
                if not final:
                    pending.append((yts, yt2s, qc))

    nc.finalize()
    return nc


def _get_nc():
    if "nc" not in _cache:
        _cache["nc"] = _build_nc()
    return _cache["nc"]


def _prep_inputs(x, latent_w, latent_b, Wd_w, Wd_b, out_w):
    """Host-side shard + layout prep. Returns list of 8 per-core input maps."""
    bf16 = ml_dtypes.bfloat16
    x = np.asarray(x, dtype=np.float32)
    latent_w = np.asarray(latent_w, dtype=np.float32)
    latent_b = np.asarray(latent_b, dtype=np.float32)
    Wd_w = np.asarray(Wd_w, dtype=np.float32)
    Wd_b = np.asarray(Wd_b, dtype=np.float32)
    out_w = np.asarray(out_w, dtype=np.float32)

    xT = np.ascontiguousarray(x.transpose(0, 2, 1)).reshape(B, 8, 128, T)
    xT = xT.astype(bf16)

    lw = np.zeros((C, 289), np.float32)
    lw[:, :288] = latent_w
    lw = lw.reshape(8, 128, 289).astype(bf16)

    lbt = np.zeros((128, 3), np.float32)
    for lt, lsz in LT:
        lbt[:lsz, lt] = latent_b[lt * 128 : lt * 128 + lsz]

    # causal masks for the 4 diagonal key tiles: mask[i][tk, tq] = tq >= i*128+tk
    tq = np.arange(512)[None, :]
    tk = np.arange(128)[:, None]
    masks = np.stack([(tq >= i * 128 + tk) for i in range(4)]).astype(np.float32)
    masks = masks.astype(bf16)
    id128 = np.eye(128, dtype=np.float32).astype(bf16)

    # per-head-group weights (shared by the two cores of each group)
    grp_maps = []
    for g in range(CPB):
        heads = [HPC * g + i for i in range(HPC)]
        wd = np.zeros((8, 128, 1152), np.float32)
        wd2 = np.zeros((8, 128, 128), np.float32)
        wdbt = np.zeros((128, 12), np.float32)
        wdbt2 = np.zeros((128, 1), np.float32)
        ow = np.zeros((8, 128, 1024), np.float32)
        ow2 = np.zeros((128, 1024), np.float32)
        for i, h in enumerate(heads):
            ow2[i * 32 : (i + 1) * 32, :] = out_w[h * 288 + 256 : h * 288 + 288, :]
            wd2[:, :, i * 32 : (i + 1) * 32] = (
                Wd_w[h][:, 256:288] / 8.0
            ).reshape(8, 128, 32)
            wdbt2[i * 32 : (i + 1) * 32, 0] = Wd_b[h][256:288] / 8.0
            wd[:, :, i * 288 : (i + 1) * 288] = (Wd_w[h] / 8.0).reshape(8, 128, 288)
            for lt, lsz in LT:
                wdbt[:lsz, i * 3 + lt] = Wd_b[h][lt * 128 : lt * 128 + lsz] / 8.0
                if lt < 2:
                    ow[i * 2 + lt, :lsz, :] = out_w[
                        h * 288 + lt * 128 : h * 288 + lt * 128 + lsz, :
                    ]
        grp_maps.append(
            {
                "wd": wd.astype(bf16),
                "wd2": wd2.astype(bf16),
                "wdbt": wdbt,
                "wdbt2": wdbt2,
                "ow": ow.astype(bf16),
                "ow2": ow2.astype(bf16),
            }
        )

    in_maps = []
    for c in range(NCORES):
        b, g = divmod(c, CPB)
        m = {
            "xT": xT[b],
            "lw": lw,
            "lbt": lbt,
            "masks": masks,
            "id128": id128,
        }
        m.update(grp_maps[g])
        in_maps.append(m)
    return in_maps


def _combine(results, out_b):
    out = np.zeros((B, T, C), np.float64)
    for c in range(NCORES):
        out[c // CPB] += results[c]["out_p"].astype(np.float64)
    out += np.asarray(out_b, dtype=np.float64)[None, None, :]
    return out.astype(np.float32)


def kernel(x, latent_w, latent_b, Wd_w, Wd_b, out_w, out_b, **kw):
    from concourse import bass_utils

    nc = _get_nc()
    in_maps = _prep_inputs(x, latent_w, latent_b, Wd_w, Wd_b, out_w)
    res = bass_utils.run_bass_kernel_spmd(nc, in_maps, core_ids=list(range(NCORES)))
    return _combine(res.results, out_b)


# revision 37
# speedup vs baseline: 1.2164x; 1.0070x over previous
"""Multi-head latent attention (MLA-style) Trainium2 kernel, 8-core SPMD.

Sharding: tensor-parallel over (batch x heads). Core c handles batch
b = c // 4 and the 4 heads 4*(c%4) .. 4*(c%4)+3:
  - kv latent (Wdkv) computed per core for its batch only
  - per-head compressed q, latent-space causal attention, and the head's
    slice of the output projection (row-sharded out_w)
  - per-core output is a PARTIAL [T, C] sum for its batch; host adds the
    4 partials per batch and the output bias.

All matmuls run in bf16 (fp32 PSUM accumulation).

Layouts (host-prepared):
  xT     [8, 128, T]      x[b].T              (c = o*128 + p)
  lw     [8, 128, 289]    latent_w, zero-padded col 288
  lbt    [128, 3]         latent_b per l-tile (fp32)
  wd     [8, 128, 1152]   Wd_w[h]/8 for the core's 4 heads, h*288+l
  wd2    [8, 128, 128]    Wd_w[h][:, 256:288]/8 stacked over 4 heads
  wdbt   [128, 12]        Wd_b[h]/8 per (h, l-tile) (fp32)
  wdbt2  [128, 1]         Wd_b[h][256:288]/8 stacked (fp32)
  ow     [8, 128, 1024]   out_w rows per (h, lt in 0..1)
  ow2    [128, 1024]      out_w l2 rows stacked over 4 heads
  masks  [4, 128, 512]    causal masks for the 4 diagonal key tiles
Output:
  out_p  [2048, 1024] fp32 partial (for the core's batch)
"""

import numpy as np
import ml_dtypes

B, T, C = 2, 2048, 1024
H, L = 16, 288
NCORES = 8
HPC = 4  # heads per core
CPB = NCORES // B  # cores per batch

# l-dimension tiles of L=288 (and the +1 sum row for the y matmul)
LT = [(0, 128), (1, 128), (2, 32)]
MT = [(0, 128), (1, 128), (2, 33)]  # y-matmul M tiles (includes sum row 288)

_cache = {}


def _build_nc():
    import concourse.bacc as bacc
    import concourse.mybir as mybir
    import concourse.tile as tile
    from concourse.bass import ts

    bf16 = mybir.dt.bfloat16
    f32 = mybir.dt.float32

    nc = bacc.Bacc("TRN2", target_bir_lowering=False, debug=True)

    d_xT = nc.dram_tensor("xT", [8, 128, T], bf16, kind="ExternalInput")
    d_lw = nc.dram_tensor("lw", [8, 128, 289], bf16, kind="ExternalInput")
    d_lbt = nc.dram_tensor("lbt", [128, 3], f32, kind="ExternalInput")
    d_wd = nc.dram_tensor("wd", [8, 128, 1152], bf16, kind="ExternalInput")
    d_wd2 = nc.dram_tensor("wd2", [8, 128, 128], bf16, kind="ExternalInput")
    d_wdbt = nc.dram_tensor("wdbt", [128, 12], f32, kind="ExternalInput")
    d_wdbt2 = nc.dram_tensor("wdbt2", [128, 1], f32, kind="ExternalInput")
    d_ow = nc.dram_tensor("ow", [8, 128, 1024], bf16, kind="ExternalInput")
    d_ow2 = nc.dram_tensor("ow2", [128, 1024], bf16, kind="ExternalInput")
    d_masks = nc.dram_tensor("masks", [4, 128, 512], bf16, kind="ExternalInput")
    d_id = nc.dram_tensor("id128", [128, 128], bf16, kind="ExternalInput")
    d_out = nc.dram_tensor("out_p", [T, C], f32, kind="ExternalOutput")

    Exp = mybir.ActivationFunctionType.Exp
    Ident = mybir.ActivationFunctionType.Identity
    Ln = mybir.ActivationFunctionType.Ln

    with tile.TileContext(nc) as tc:
        with (
            tc.tile_pool(name="const", bufs=1) as cpool,
            tc.tile_pool(name="xp", bufs=1) as xpool,
            tc.tile_pool(name="kvp", bufs=1) as kvpool,
            tc.tile_pool(name="qp", bufs=2) as qpool,
            tc.tile_pool(name="ep", bufs=4) as epool,
            tc.tile_pool(name="yp", bufs=2) as ypool,
            tc.tile_pool(name="rp", bufs=2) as rpool,
            tc.tile_pool(name="op", bufs=3) as opool,
            tc.tile_pool(name="ps_y", bufs=1, space="PSUM") as ps_y,
            tc.tile_pool(name="ps_s", bufs=3, space="PSUM") as ps_s,
            tc.tile_pool(name="ps_m", bufs=2, space="PSUM") as ps_m,
        ):
            # ---- persistent weights ----
            # latent_w first: the kvT matmuls only need lw + the first x
            # chunk, so the PE can start ~10us earlier
            lw_sb = cpool.tile([128, 8, 289], bf16, name="lw_sb")
            for kc in range(8):
                # split across the two HWDGE queues to halve the startup
                # serial chain (kv matmuls consume kc in order)
                eng = nc.sync if kc % 2 == 0 else nc.scalar
                eng.dma_start(lw_sb[:, kc, :], d_lw[kc])

            lbt_sb = cpool.tile([128, 3], f32, name="lbt_sb")
            nc.sync.dma_start(lbt_sb[:], d_lbt[:])
            id_sb = cpool.tile([128, 128], bf16, name="id_sb")
            nc.sync.dma_start(id_sb[:], d_id[:])
            wd_sb = cpool.tile([128, 8, 1152], bf16, name="wd_sb")
            wd2_sb = cpool.tile([128, 8, 128], bf16, name="wd2_sb")
            wdbt_sb = cpool.tile([128, 12], f32, name="wdbt_sb")
            wdbt2_sb = cpool.tile([128, 1], f32, name="wdbt2_sb")
            ow_sb = cpool.tile([128, 8, 1024], bf16, name="ow_sb")
            ow2_sb = cpool.tile([128, 1024], bf16, name="ow2_sb")
            masks_sb = cpool.tile([128, 4, 512], bf16, name="masks_sb")

            def load_weights():
                for kc in range(8):
                    eng = nc.sync if kc % 2 == 0 else nc.scalar
                    eng.dma_start(wd_sb[:, kc, :], d_wd[kc])
                    eng.dma_start(wd2_sb[:, kc, :], d_wd2[kc])
                nc.sync.dma_start(wdbt_sb[:], d_wdbt[:])
                nc.scalar.dma_start(wdbt2_sb[:], d_wdbt2[:])
                for i in range(8):
                    eng = nc.sync if i % 2 == 0 else nc.scalar
                    eng.dma_start(ow_sb[:, i, :], d_ow[i])
                nc.sync.dma_start(ow2_sb[:], d_ow2[:])
                for i in range(4):
                    eng = nc.sync if i % 2 == 0 else nc.scalar
                    eng.dma_start(masks_sb[:, i, :], d_masks[i])

            # deferred out-projection: (yts, qc) emitted one head late so
            # the PE queue never blocks on the normalize chain
            pending = []

            def emit_outproj():
                yts, yt2s, pqc = pending.pop()
                for blk in range(4):
                    osb = opool.tile([128, 1024], f32, name="osb", tag="osb")
                    for cc in range(2):
                        po = ps_m.tile([128, 512], f32, name="ps_o", tag="m")
                        for h in range(HPC):
                            for lt in (0, 1):
                                nc.tensor.matmul(
                                    po,
                                    yts[h][:, lt, ts(blk, 128)],
                                    ow_sb[:, h * 2 + lt, ts(cc, 512)],
                                    start=(h == 0 and lt == 0),
                                    stop=False,
                                )
                        # all 4 heads' l2 blocks stacked into one K=128 matmul
                        nc.tensor.matmul(
                            po,
                            yt2s[:, ts(blk, 128)],
                            ow2_sb[:, ts(cc, 512)],
                            start=False,
                            stop=True,
                        )
                        nc.vector.tensor_copy(osb[:, ts(cc, 512)], po[:])
                    row0 = pqc * 512 + blk * 128
                    nc.sync.dma_start(d_out[row0 : row0 + 128, :], osb[:])

            # the last chunk's out-projection is emitted in two stages
            # (heads 0..2 overlap the last head's attention) to shrink the
            # end-of-kernel tail
            def emit_final_front(yts):
                osbs = []
                for blk in range(4):
                    osb = opool.tile(
                        [128, 1024], f32, name="osbf", tag="osbf", bufs=4
                    )
                    for cc in range(2):
                        po = ps_m.tile([128, 512], f32, name="ps_o", tag="m")
                        for h in range(HPC - 1):
                            for lt in (0, 1):
                                nc.tensor.matmul(
                                    po,
                                    yts[h][:, lt, ts(blk, 128)],
                                    ow_sb[:, h * 2 + lt, ts(cc, 512)],
                                    start=(h == 0 and lt == 0),
                                    stop=(h == HPC - 2 and lt == 1),
                                )
                        nc.vector.tensor_copy(osb[:, ts(cc, 512)], po[:])
                    osbs.append(osb)
                return osbs

            def emit_final_back(yt, yt2f, osbs, pqc):
                hl = HPC - 1
                for blk in range(4):
                    for cc in range(2):
                        po = ps_m.tile([128, 512], f32, name="ps_o", tag="m")
                        for lt in (0, 1):
                            nc.tensor.matmul(
                                po,
                                yt[:, lt, ts(blk, 128)],
                                ow_sb[:, hl * 2 + lt, ts(cc, 512)],
                                start=(lt == 0),
                                stop=False,
                            )
                        nc.tensor.matmul(
                            po,
                            yt2f[:, ts(blk, 128)],
                            ow2_sb[:, ts(cc, 512)],
                            start=False,
                            stop=True,
                        )
                        nc.vector.tensor_add(
                            osbs[blk][:, ts(cc, 512)],
                            po[:],
                            osbs[blk][:, ts(cc, 512)],
                        )
                    row0 = pqc * 512 + blk * 128
                    nc.sync.dma_start(d_out[row0 : row0 + 128, :], osbs[blk][:])

            # ---- load x^T, per 512-chunk (SWDGE queues, parallel to the
            # HWDGE weight loads) ----
            xts = []
            for tch in range(4):
                xt = xpool.tile([128, 8, 512], bf16, name="xt", tag=f"xT{tch}")
                for o in range(8):
                    nc.gpsimd.dma_start(xt[:, o, :], d_xT[o][:, ts(tch, 512)])
                xts.append(xt)
            load_weights()

            # ---- kvT = (x @ latent_w + latent_b)^T : [l, t], per chunk;
            #      kv_aug[t, 0:289] = [kv | 1] via PE transpose ----
            def compute_kv(xtile, ktag):
                kvt = kvpool.tile([128, 3, 512], bf16, name="kvt", tag=f"kvT{ktag}")
                for lt, lsz in LT:
                    pq = ps_s.tile([128, 512], f32, name="ps_kv", tag="s")
                    for kc in range(8):
                        nc.tensor.matmul(
                            pq[:lsz],
                            lw_sb[:, kc, lt * 128 : lt * 128 + lsz],
                            xtile[:, kc, :],
                            start=(kc == 0),
                            stop=(kc == 7),
                        )
                    nc.scalar.activation(
                        kvt[:lsz, lt, :],
                        pq[:lsz],
                        Ident,
                        bias=lbt_sb[:lsz, lt : lt + 1],
                    )

                # kv-l2 relaid out so adjacent t-tiles sit at partition
                # offsets 0/32, enabling paired (concurrent) K=32 matmuls
                kv2p = kvpool.tile([64, 2, 128], bf16, name="kv2p", tag=f"kv2p{ktag}")
                for j in range(4):
                    nc.sync.dma_start(
                        kv2p[32 * (j % 2) : 32 * (j % 2) + 32, j // 2, :],
                        kvt[:32, 2, ts(j, 128)],
                    )

                kva = kvpool.tile([128, 4, 289], bf16, name="kva", tag=f"kva{ktag}")
                for tt in range(4):
                    nc.vector.memset(kva[:, tt, 288:289], 1.0)
                    for lt, lsz in LT:
                        pt = ps_m.tile([128, 512], bf16, name="ps_t", tag="m")
                        nc.tensor.transpose(
                            pt[:, :lsz],
                            kvt[:lsz, lt, ts(tt, 128)],
                            id_sb[:lsz, :lsz],
                        )
                        nc.vector.tensor_copy(
                            kva[:, tt, lt * 128 : lt * 128 + lsz], pt[:, :lsz]
                        )
                return kvt, kv2p, kva

            kvts, kv2ps, kvas = [], [], []
            for tch in range(4):
                kvt, kv2p, kva = compute_kv(xts[tch], tch)
                kvts.append(kvt)
                kv2ps.append(kv2p)
                kvas.append(kva)

            # ---- attention per (chunk, head) ----
            for qc in range(4):
                final = qc == 3
                yts = []
                yt2s = ypool.tile([128, 512], bf16, name="yt2s", tag="yt2")

                # all 4 heads' l2 (l=256..287) q-projection stacked into
                # one M=128 matmul group; each head's half is then
                # DMA-replicated at partition offsets 0/32 so the paired
                # scores matmul K ranges line up
                pq2 = ps_s.tile([128, 512], f32, name="ps_q2", tag="s")
                for kc in range(8):
                    nc.tensor.matmul(
                        pq2,
                        wd2_sb[:, kc, :],
                        xts[qc][:, kc, :],
                        start=(kc == 0),
                        stop=(kc == 7),
                    )
                qt2w = qpool.tile([128, 512], bf16, name="qt2w", tag="qt2w")
                nc.scalar.activation(
                    qt2w[:], pq2[:], Ident, bias=wdbt2_sb[:, 0:1]
                )
                # per-head pair-replica: qrep[0:32,h]=qrep[32:64,h]=q2_h
                qrep = qpool.tile([64, 4, 512], bf16, name="qrep", tag="qrep")
                for h in range(HPC):
                    nc.sync.dma_start(qrep[0:32, h, :], qt2w[32 * h : 32 * h + 32, :])
                    nc.gpsimd.dma_start(
                        qrep[32:64, h, :], qt2w[32 * h : 32 * h + 32, :]
                    )

                for h in range(HPC):
                    # q^T chunk [l, 512] (scale 1/8 folded into wd)
                    qt = qpool.tile([128, 2, 512], bf16, name="qt", tag="qt")
                    for lt in (0, 1):
                        pq = ps_s.tile([128, 512], f32, name="ps_q", tag="s")
                        for kc in range(8):
                            nc.tensor.matmul(
                                pq,
                                wd_sb[:, kc, h * 288 + lt * 128 :][:, :128],
                                xts[qc][:, kc, :],
                                start=(kc == 0),
                                stop=(kc == 7),
                            )
                        nc.scalar.activation(
                            qt[:, lt, :],
                            pq[:],
                            Ident,
                            bias=wdbt_sb[:, h * 3 + lt : h * 3 + lt + 1],
                        )

                    # scores^T -> exp -> (mask) -> y accumulation
                    py = [
                        ps_y.tile([128, 512], f32, name=f"ps_y{mt}", tag=f"y{mt}")
                        for mt, _ in MT
                    ]
                    ntk = qc * 4 + 4

                    def emit_y(tk, et, c0):
                        for mt, msz in MT:
                            nc.tensor.matmul(
                                py[mt][:msz, c0:],
                                kvas[tk // 4][:, tk % 4, mt * 128 :][:, :msz],
                                et[:, c0:],
                                start=(tk == 0),
                                stop=(tk == ntk - 1),
                            )

                    # scores/exp pipelined one pair ahead of the y matmuls
                    # so the PE queue never blocks on the ACT exp; the two
                    # K=32 l2 matmuls of each pair run in concurrent PE
                    # row groups (partition offsets 0 / 32)
                    pend = []
                    for pr in range(ntk // 2):
                        pair = []
                        for tk in (2 * pr, 2 * pr + 1):
                            # diagonal tiles: only columns >= c0 unmasked
                            c0 = max(0, (tk - qc * 4) * 128)
                            pss = ps_s.tile(
                                [128, 512], f32, name="ps_s", tag="s"
                            )
                            for lt in (0, 1):
                                nc.tensor.matmul(
                                    pss[:, c0:],
                                    kvts[tk // 4][:, lt, ts(tk % 4, 128)],
                                    qt[:, lt, c0:],
                                    start=(lt == 0),
                                    stop=False,
                                )
                            pair.append((tk, pss, c0))
                        for off, (tk, pss, c0) in zip((0, 32), pair):
                            nc.tensor.matmul(
                                pss[:, c0:],
                                kv2ps[tk // 4][
                                    off : off + 32, (tk % 4) // 2, :
                                ],
                                qrep[off : off + 32, h, c0:],
                                start=False,
                                stop=True,
                            )
                        for tk, pss, c0 in pair:
                            et = epool.tile(
                                [128, 512], bf16, name="et", tag="et"
                            )
                            nc.scalar.activation(et[:, c0:], pss[:, c0:], Exp)
                            i = tk - qc * 4
                            if i >= 0:
                                # mask is nontrivial only in the i-th
                                # 128-column block
                                nc.vector.tensor_mul(
                                    et[:, c0 : c0 + 128],
                                    et[:, c0 : c0 + 128],
                                    masks_sb[:, i, c0 : c0 + 128],
                                )
                            pend.append((tk, et, c0))
                        while len(pend) > 2:
                            emit_y(*pend.pop(0))
                        if final and h == HPC - 1 and pr == 3:
                            # earlier heads' deferred out-projection, emitted
                            # here so its matmuls enter the PE queue well
                            # after their normalize chains have completed
                            final_osbs = emit_final_front(yts)
                    for e in pend:
                        emit_y(*e)

                    # drain the PSUM banks immediately (unnormalized), so the
                    # next head's matmuls never wait on the normalize chain
                    lnw = rpool.tile([1, 512], f32, name="lnw", tag="lnw")
                    nc.scalar.activation(lnw[:], py[2][32:33, :], Ln)
                    yu = ypool.tile(
                        [128, 2, 512], bf16, name="yu", tag=f"yu{h}", bufs=1
                    )
                    for lt in (0, 1):
                        nc.vector.tensor_copy(yu[:, lt, :], py[lt][:])
                    yu2 = rpool.tile([32, 512], bf16, name="yu2", tag=f"yu2{h}")
                    nc.vector.tensor_copy(yu2[:], py[2][:32])

                    # prev-head out-projection enqueues (PE + DVE copies)
                    # ahead of the normalize tail in the engine FIFOs
                    if pending:
                        emit_outproj()

                    # r = exp(-ln(sum)) = 1/sum, entirely on the scalar
                    # engine: keeps the serial 3.3us DVE reciprocal out of
                    # the DVE FIFO that the yt muls (and thus the deferred
                    # out-projection) queue behind
                    r_sb = rpool.tile([1, 512], f32, name="r_sb", tag="r")
                    nc.scalar.activation(r_sb[:], lnw[:], Exp, scale=-1.0)
                    rb_sb = rpool.tile([128, 512], f32, name="rb_sb", tag="rb")
                    nc.gpsimd.partition_broadcast(rb_sb[:], r_sb[:1, :])
                    yt = ypool.tile([128, 2, 512], bf16, name="yt", tag=f"yt{h}")
                    for lt in (0, 1):
                        nc.vector.tensor_mul(yt[:, lt, :], yu[:, lt, :], rb_sb[:])
                    nc.vector.tensor_mul(
                        yt2s[h * 32 : (h + 1) * 32, :], yu2[:], rb_sb[:32]
                    )
                    yts.append(yt)

                    if final and h == HPC - 1:
                        emit_final_back(yt, yt2s, final_osbs, qc)
                if not final:
                    pending.append((yts, yt2s, qc))

    nc.finalize()
    return nc


def _get_nc():
    if "nc" not in _cache:
        _cache["nc"] = _build_nc()
    return _cache["nc"]


def _prep_inputs(x, latent_w, latent_b, Wd_w, Wd_b, out_w):
    """Host-side shard + layout prep. Returns list of 8 per-core input maps."""
    bf16 = ml_dtypes.bfloat16
    x = np.asarray(x, dtype=np.float32)
    latent_w = np.asarray(latent_w, dtype=np.float32)
    latent_b = np.asarray(latent_b, dtype=np.float32)
    Wd_w = np.asarray(Wd_w, dtype=np.float32)
    Wd_b = np.asarray(Wd_b, dtype=np.float32)
    out_w = np.asarray(out_w, dtype=np.float32)

    xT = np.ascontiguousarray(x.transpose(0, 2, 1)).reshape(B, 8, 128, T)
    xT = xT.astype(bf16)

    lw = np.zeros((C, 289), np.float32)
    lw[:, :288] = latent_w
    lw = lw.reshape(8, 128, 289).astype(bf16)

    lbt = np.zeros((128, 3), np.float32)
    for lt, lsz in LT:
        lbt[:lsz, lt] = latent_b[lt * 128 : lt * 128 + lsz]

    # causal masks for the 4 diagonal key tiles: mask[i][tk, tq] = tq >= i*128+tk
    tq = np.arange(512)[None, :]
    tk = np.arange(128)[:, None]
    masks = np.stack([(tq >= i * 128 + tk) for i in range(4)]).astype(np.float32)
    masks = masks.astype(bf16)
    id128 = np.eye(128, dtype=np.float32).astype(bf16)

    # per-head-group weights (shared by the two cores of each group)
    grp_maps = []
    for g in range(CPB):
        heads = [HPC * g + i for i in range(HPC)]
        wd = np.zeros((8, 128, 1152), np.float32)
        wd2 = np.zeros((8, 128, 128), np.float32)
        wdbt = np.zeros((128, 12), np.float32)
        wdbt2 = np.zeros((128, 1), np.float32)
        ow = np.zeros((8, 128, 1024), np.float32)
        ow2 = np.zeros((128, 1024), np.float32)
        for i, h in enumerate(heads):
            ow2[i * 32 : (i + 1) * 32, :] = out_w[h * 288 + 256 : h * 288 + 288, :]
            wd2[:, :, i * 32 : (i + 1) * 32] = (
                Wd_w[h][:, 256:288] / 8.0
            ).reshape(8, 128, 32)
            wdbt2[i * 32 : (i + 1) * 32, 0] = Wd_b[h][256:288] / 8.0
            wd[:, :, i * 288 : (i + 1) * 288] = (Wd_w[h] / 8.0).reshape(8, 128, 288)
            for lt, lsz in LT:
                wdbt[:lsz, i * 3 + lt] = Wd_b[h][lt * 128 : lt * 128 + lsz] / 8.0
                if lt < 2:
                    ow[i * 2 + lt, :lsz, :] = out_w[
                        h * 288 + lt * 128 : h * 288 + lt * 128 + lsz, :
                    ]
        grp_maps.append(
            {
                "wd": wd.astype(bf16),
                "wd2": wd2.astype(bf16),
                "wdbt": wdbt,
                "wdbt2": wdbt2,
                "ow": ow.astype(bf16),
                "ow2": ow2.astype(bf16),
            }
        )

    in_maps = []
    for c in range(NCORES):
        b, g = divmod(c, CPB)
        m = {
            "xT": xT[b],
            "lw": lw,
            "lbt": lbt,
            "masks": masks,
            "id128": id128,
        }
        m.update(grp_maps[g])
        in_maps.append(m)
    return in_maps


def _combine(results, out_b):
    out = np.zeros((B, T, C), np.float64)
    for c in range(NCORES):
        out[c // CPB] += results[c]["out_p"].astype(np.float64)
    out += np.asarray(out_b, dtype=np.float64)[None, None, :]
    return out.astype(np.float32)


def kernel(x, latent_w, latent_b, Wd_w, Wd_b, out_w, out_b, **kw):
    from concourse import bass_utils

    nc = _get_nc()
    in_maps = _prep_inputs(x, latent_w, latent_b, Wd_w, Wd_b, out_w)
    res = bass_utils.run_bass_kernel_spmd(nc, in_maps, core_ids=list(range(NCORES)))
    return _combine(res.results, out_b)


# revision 38
# speedup vs baseline: 1.2585x; 1.0346x over previous
"""Multi-head latent attention (MLA-style) Trainium2 kernel, 8-core SPMD.

Sharding: tensor-parallel over (batch x heads). Core c handles batch
b = c // 4 and the 4 heads 4*(c%4) .. 4*(c%4)+3:
  - kv latent (Wdkv) computed per core for its batch only
  - per-head compressed q, latent-space causal attention, and the head's
    slice of the output projection (row-sharded out_w)
  - per-core output is a PARTIAL [T, C] sum for its batch; host adds the
    4 partials per batch and the output bias.

All matmuls run in bf16 (fp32 PSUM accumulation).

Layouts (host-prepared):
  xT     [8, 128, T]      x[b].T              (c = o*128 + p)
  lw     [8, 128, 289]    latent_w, zero-padded col 288
  lbt    [128, 3]         latent_b per l-tile (fp32)
  wd     [8, 128, 1152]   Wd_w[h]/8 for the core's 4 heads, h*288+l
  wd2    [8, 128, 128]    Wd_w[h][:, 256:288]/8 stacked over 4 heads
  wdbt   [128, 12]        Wd_b[h]/8 per (h, l-tile) (fp32)
  wdbt2  [128, 1]         Wd_b[h][256:288]/8 stacked (fp32)
  ow     [8, 128, 1024]   out_w rows per (h, lt in 0..1)
  ow2    [128, 1024]      out_w l2 rows stacked over 4 heads
  masks  [4, 128, 512]    causal masks for the 4 diagonal key tiles
Output:
  out_p  [2048, 1024] fp32 partial (for the core's batch)
"""

import numpy as np
import ml_dtypes

B, T, C = 2, 2048, 1024
H, L = 16, 288
NCORES = 8
HPC = 4  # heads per core
CPB = NCORES // B  # cores per batch

# l-dimension tiles of L=288 (and the +1 sum row for the y matmul)
LT = [(0, 128), (1, 128), (2, 32)]
MT = [(0, 128), (1, 128), (2, 33)]  # y-matmul M tiles (includes sum row 288)

_cache = {}


def _build_nc():
    import concourse.bacc as bacc
    import concourse.mybir as mybir
    import concourse.tile as tile
    from concourse.bass import ts

    bf16 = mybir.dt.bfloat16
    f32 = mybir.dt.float32

    nc = bacc.Bacc("TRN2", target_bir_lowering=False, debug=True)

    d_xT = nc.dram_tensor("xT", [8, 128, T], bf16, kind="ExternalInput")
    d_lw = nc.dram_tensor("lw", [8, 128, 289], bf16, kind="ExternalInput")
    d_lbt = nc.dram_tensor("lbt", [128, 3], f32, kind="ExternalInput")
    d_wd = nc.dram_tensor("wd", [8, 128, 1152], bf16, kind="ExternalInput")
    d_wd2 = nc.dram_tensor("wd2", [8, 128, 128], bf16, kind="ExternalInput")
    d_wdbt = nc.dram_tensor("wdbt", [128, 12], f32, kind="ExternalInput")
    d_wdbt2 = nc.dram_tensor("wdbt2", [128, 1], f32, kind="ExternalInput")
    d_ow = nc.dram_tensor("ow", [8, 128, 1024], bf16, kind="ExternalInput")
    d_ow2 = nc.dram_tensor("ow2", [128, 1024], bf16, kind="ExternalInput")
    d_masks = nc.dram_tensor("masks", [4, 128, 512], bf16, kind="ExternalInput")
    d_id = nc.dram_tensor("id128", [128, 128], bf16, kind="ExternalInput")
    d_out = nc.dram_tensor("out_p", [T, C], f32, kind="ExternalOutput")

    Exp = mybir.ActivationFunctionType.Exp
    Ident = mybir.ActivationFunctionType.Identity
    Ln = mybir.ActivationFunctionType.Ln

    with tile.TileContext(nc) as tc:
        with (
            tc.tile_pool(name="const", bufs=1) as cpool,
            tc.tile_pool(name="xp", bufs=1) as xpool,
            tc.tile_pool(name="kvp", bufs=1) as kvpool,
            tc.tile_pool(name="qp", bufs=2) as qpool,
            tc.tile_pool(name="ep", bufs=4) as epool,
            tc.tile_pool(name="yp", bufs=2) as ypool,
            tc.tile_pool(name="rp", bufs=2) as rpool,
            tc.tile_pool(name="op", bufs=3) as opool,
            tc.tile_pool(name="ps_y", bufs=1, space="PSUM") as ps_y,
            tc.tile_pool(name="ps_s", bufs=3, space="PSUM") as ps_s,
            tc.tile_pool(name="ps_m", bufs=2, space="PSUM") as ps_m,
        ):
            # ---- persistent weights ----
            # latent_w first: the kvT matmuls only need lw + the first x
            # chunk, so the PE can start ~10us earlier
            lw_sb = cpool.tile([128, 8, 289], bf16, name="lw_sb")
            for kc in range(8):
                # split across the two HWDGE queues to halve the startup
                # serial chain (kv matmuls consume kc in order)
                eng = nc.sync if kc % 2 == 0 else nc.scalar
                eng.dma_start(lw_sb[:, kc, :], d_lw[kc])

            lbt_sb = cpool.tile([128, 3], f32, name="lbt_sb")
            nc.sync.dma_start(lbt_sb[:], d_lbt[:])
            id_sb = cpool.tile([128, 128], bf16, name="id_sb")
            nc.sync.dma_start(id_sb[:], d_id[:])
            wd_sb = cpool.tile([128, 8, 1152], bf16, name="wd_sb")
            wd2_sb = cpool.tile([128, 8, 128], bf16, name="wd2_sb")
            wdbt_sb = cpool.tile([128, 12], f32, name="wdbt_sb")
            wdbt2_sb = cpool.tile([128, 1], f32, name="wdbt2_sb")
            ow_sb = cpool.tile([128, 8, 1024], bf16, name="ow_sb")
            ow2_sb = cpool.tile([128, 1024], bf16, name="ow2_sb")
            masks_sb = cpool.tile([128, 4, 512], bf16, name="masks_sb")

            def load_weights():
                for kc in range(8):
                    nc.sync.dma_start(wd_sb[:, kc, :], d_wd[kc])
                    nc.sync.dma_start(wd2_sb[:, kc, :], d_wd2[kc])
                nc.sync.dma_start(wdbt_sb[:], d_wdbt[:])
                nc.sync.dma_start(wdbt2_sb[:], d_wdbt2[:])
                for i in range(8):
                    nc.sync.dma_start(ow_sb[:, i, :], d_ow[i])
                nc.sync.dma_start(ow2_sb[:], d_ow2[:])
                for i in range(4):
                    nc.sync.dma_start(masks_sb[:, i, :], d_masks[i])

            # deferred out-projection: (yts, qc) emitted one head late so
            # the PE queue never blocks on the normalize chain
            pending = []

            def emit_outproj():
                yts, yt2s, pqc = pending.pop()
                for blk in range(4):
                    osb = opool.tile([128, 1024], f32, name="osb", tag="osb")
                    for cc in range(2):
                        po = ps_m.tile([128, 512], f32, name="ps_o", tag="m")
                        for h in range(HPC):
                            for lt in (0, 1):
                                nc.tensor.matmul(
                                    po,
                                    yts[h][:, lt, ts(blk, 128)],
                                    ow_sb[:, h * 2 + lt, ts(cc, 512)],
                                    start=(h == 0 and lt == 0),
                                    stop=False,
                                )
                        # all 4 heads' l2 blocks stacked into one K=128 matmul
                        nc.tensor.matmul(
                            po,
                            yt2s[:, ts(blk, 128)],
                            ow2_sb[:, ts(cc, 512)],
                            start=False,
                            stop=True,
                        )
                        nc.vector.tensor_copy(osb[:, ts(cc, 512)], po[:])
                    row0 = pqc * 512 + blk * 128
                    nc.sync.dma_start(d_out[row0 : row0 + 128, :], osb[:])

            # the last chunk's out-projection is emitted in two stages
            # (heads 0..2 overlap the last head's attention) to shrink the
            # end-of-kernel tail
            def emit_final_front(yts):
                osbs = []
                for blk in range(4):
                    osb = opool.tile(
                        [128, 1024], f32, name="osbf", tag="osbf", bufs=4
                    )
                    for cc in range(2):
                        po = ps_m.tile([128, 512], f32, name="ps_o", tag="m")
                        for h in range(HPC - 1):
                            for lt in (0, 1):
                                nc.tensor.matmul(
                                    po,
                                    yts[h][:, lt, ts(blk, 128)],
                                    ow_sb[:, h * 2 + lt, ts(cc, 512)],
                                    start=(h == 0 and lt == 0),
                                    stop=(h == HPC - 2 and lt == 1),
                                )
                        nc.vector.tensor_copy(osb[:, ts(cc, 512)], po[:])
                    osbs.append(osb)
                return osbs

            def emit_final_back(yt, yt2f, osbs, pqc):
                hl = HPC - 1
                for blk in range(4):
                    for cc in range(2):
                        po = ps_m.tile([128, 512], f32, name="ps_o", tag="m")
                        for lt in (0, 1):
                            nc.tensor.matmul(
                                po,
                                yt[:, lt, ts(blk, 128)],
                                ow_sb[:, hl * 2 + lt, ts(cc, 512)],
                                start=(lt == 0),
                                stop=False,
                            )
                        nc.tensor.matmul(
                            po,
                            yt2f[:, ts(blk, 128)],
                            ow2_sb[:, ts(cc, 512)],
                            start=False,
                            stop=True,
                        )
                        nc.vector.tensor_add(
                            osbs[blk][:, ts(cc, 512)],
                            po[:],
                            osbs[blk][:, ts(cc, 512)],
                        )
                    row0 = pqc * 512 + blk * 128
                    nc.sync.dma_start(d_out[row0 : row0 + 128, :], osbs[blk][:])

            # ---- load x^T, per 512-chunk (SWDGE queues, parallel to the
            # HWDGE weight loads) ----
            xts = []
            for tch in range(4):
                xt = xpool.tile([128, 8, 512], bf16, name="xt", tag=f"xT{tch}")
                for o in range(8):
                    nc.gpsimd.dma_start(xt[:, o, :], d_xT[o][:, ts(tch, 512)])
                xts.append(xt)
            load_weights()

            # ---- kvT = (x @ latent_w + latent_b)^T : [l, t], per chunk;
            #      kv_aug[t, 0:289] = [kv | 1] via PE transpose ----
            def compute_kv(xtile, ktag):
                kvt = kvpool.tile([128, 3, 512], bf16, name="kvt", tag=f"kvT{ktag}")
                for lt, lsz in LT:
                    pq = ps_s.tile([128, 512], f32, name="ps_kv", tag="s")
                    for kc in range(8):
                        nc.tensor.matmul(
                            pq[:lsz],
                            lw_sb[:, kc, lt * 128 : lt * 128 + lsz],
                            xtile[:, kc, :],
                            start=(kc == 0),
                            stop=(kc == 7),
                        )
                    nc.scalar.activation(
                        kvt[:lsz, lt, :],
                        pq[:lsz],
                        Ident,
                        bias=lbt_sb[:lsz, lt : lt + 1],
                    )

                # kv-l2 relaid out so adjacent t-tiles sit at partition
                # offsets 0/32, enabling paired (concurrent) K=32 matmuls
                kv2p = kvpool.tile([64, 2, 128], bf16, name="kv2p", tag=f"kv2p{ktag}")
                for j in range(4):
                    nc.sync.dma_start(
                        kv2p[32 * (j % 2) : 32 * (j % 2) + 32, j // 2, :],
                        kvt[:32, 2, ts(j, 128)],
                    )

                kva = kvpool.tile([128, 4, 289], bf16, name="kva", tag=f"kva{ktag}")
                for tt in range(4):
                    nc.vector.memset(kva[:, tt, 288:289], 1.0)
                    for lt, lsz in LT:
                        pt = ps_m.tile([128, 512], bf16, name="ps_t", tag="m")
                        nc.tensor.transpose(
                            pt[:, :lsz],
                            kvt[:lsz, lt, ts(tt, 128)],
                            id_sb[:lsz, :lsz],
                        )
                        nc.vector.tensor_copy(
                            kva[:, tt, lt * 128 : lt * 128 + lsz], pt[:, :lsz]
                        )
                return kvt, kv2p, kva

            kvts, kv2ps, kvas = [], [], []
            for tch in range(4):
                kvt, kv2p, kva = compute_kv(xts[tch], tch)
                kvts.append(kvt)
                kv2ps.append(kv2p)
                kvas.append(kva)

            # ---- attention per (chunk, head) ----
            for qc in range(4):
                final = qc == 3
                yts = []
                yt2s = ypool.tile([128, 512], bf16, name="yt2s", tag="yt2")

                # all 4 heads' l2 (l=256..287) q-projection stacked into
                # one M=128 matmul group; each head's half is then
                # DMA-replicated at partition offsets 0/32 so the paired
                # scores matmul K ranges line up
                pq2 = ps_s.tile([128, 512], f32, name="ps_q2", tag="s")
                for kc in range(8):
                    nc.tensor.matmul(
                        pq2,
                        wd2_sb[:, kc, :],
                        xts[qc][:, kc, :],
                        start=(kc == 0),
                        stop=(kc == 7),
                    )
                qt2w = qpool.tile([128, 512], bf16, name="qt2w", tag="qt2w")
                nc.scalar.activation(
                    qt2w[:], pq2[:], Ident, bias=wdbt2_sb[:, 0:1]
                )
                # per-head pair-replica: qrep[0:32,h]=qrep[32:64,h]=q2_h
                qrep = qpool.tile([64, 4, 512], bf16, name="qrep", tag="qrep")
                for h in range(HPC):
                    nc.sync.dma_start(qrep[0:32, h, :], qt2w[32 * h : 32 * h + 32, :])
                    nc.gpsimd.dma_start(
                        qrep[32:64, h, :], qt2w[32 * h : 32 * h + 32, :]
                    )

                for h in range(HPC):
                    # q^T chunk [l, 512] (scale 1/8 folded into wd)
                    qt = qpool.tile([128, 2, 512], bf16, name="qt", tag="qt")
                    for lt in (0, 1):
                        pq = ps_s.tile([128, 512], f32, name="ps_q", tag="s")
                        for kc in range(8):
                            nc.tensor.matmul(
                                pq,
                                wd_sb[:, kc, h * 288 + lt * 128 :][:, :128],
                                xts[qc][:, kc, :],
                                start=(kc == 0),
                                stop=(kc == 7),
                            )
                        nc.scalar.activation(
                            qt[:, lt, :],
                            pq[:],
                            Ident,
                            bias=wdbt_sb[:, h * 3 + lt : h * 3 + lt + 1],
                        )

                    # scores^T -> exp -> (mask) -> y accumulation
                    py = [
                        ps_y.tile([128, 512], f32, name=f"ps_y{mt}", tag=f"y{mt}")
                        for mt, _ in MT
                    ]
                    ntk = qc * 4 + 4

                    def emit_y(tk, et, c0):
                        for mt, msz in MT:
                            nc.tensor.matmul(
                                py[mt][:msz, c0:],
                                kvas[tk // 4][:, tk % 4, mt * 128 :][:, :msz],
                                et[:, c0:],
                                start=(tk == 0),
                                stop=(tk == ntk - 1),
                            )

                    # scores/exp pipelined one pair ahead of the y matmuls
                    # so the PE queue never blocks on the ACT exp; the two
                    # K=32 l2 matmuls of each pair run in concurrent PE
                    # row groups (partition offsets 0 / 32)
                    pend = []
                    for pr in range(ntk // 2):
                        pair = []
                        for tk in (2 * pr, 2 * pr + 1):
                            # diagonal tiles: only columns >= c0 unmasked
                            c0 = max(0, (tk - qc * 4) * 128)
                            pss = ps_s.tile(
                                [128, 512], f32, name="ps_s", tag="s"
                            )
                            for lt in (0, 1):
                                nc.tensor.matmul(
                                    pss[:, c0:],
                                    kvts[tk // 4][:, lt, ts(tk % 4, 128)],
                                    qt[:, lt, c0:],
                                    start=(lt == 0),
                                    stop=False,
                                )
                            pair.append((tk, pss, c0))
                        for off, (tk, pss, c0) in zip((0, 32), pair):
                            nc.tensor.matmul(
                                pss[:, c0:],
                                kv2ps[tk // 4][
                                    off : off + 32, (tk % 4) // 2, :
                                ],
                                qrep[off : off + 32, h, c0:],
                                start=False,
                                stop=True,
                            )
                        for tk, pss, c0 in pair:
                            et = epool.tile(
                                [128, 512], bf16, name="et", tag="et"
                            )
                            nc.scalar.activation(et[:, c0:], pss[:, c0:], Exp)
                            i = tk - qc * 4
                            if i >= 0:
                                # mask is nontrivial only in the i-th
                                # 128-column block
                                nc.vector.tensor_mul(
                                    et[:, c0 : c0 + 128],
                                    et[:, c0 : c0 + 128],
                                    masks_sb[:, i, c0 : c0 + 128],
                                )
                            pend.append((tk, et, c0))
                        while len(pend) > 2:
                            emit_y(*pend.pop(0))
                        if final and h == HPC - 1 and pr == 3:
                            # earlier heads' deferred out-projection, emitted
                            # here so its matmuls enter the PE queue well
                            # after their normalize chains have completed
                            final_osbs = emit_final_front(yts)
                    for e in pend:
                        emit_y(*e)

                    # drain the PSUM banks immediately (unnormalized), so the
                    # next head's matmuls never wait on the normalize chain
                    lnw = rpool.tile([1, 512], f32, name="lnw", tag="lnw")
                    nc.scalar.activation(lnw[:], py[2][32:33, :], Ln)
                    yu = ypool.tile(
                        [128, 2, 512], bf16, name="yu", tag=f"yu{h}", bufs=1
                    )
                    for lt in (0, 1):
                        nc.vector.tensor_copy(yu[:, lt, :], py[lt][:])
                    yu2 = rpool.tile([32, 512], bf16, name="yu2", tag=f"yu2{h}")
                    nc.vector.tensor_copy(yu2[:], py[2][:32])

                    # prev-head out-projection enqueues (PE + DVE copies)
                    # ahead of the normalize tail in the engine FIFOs
                    if pending:
                        emit_outproj()

                    # r = exp(-ln(sum)) = 1/sum, entirely on the scalar
                    # engine: keeps the serial 3.3us DVE reciprocal out of
                    # the DVE FIFO that the yt muls (and thus the deferred
                    # out-projection) queue behind
                    r_sb = rpool.tile([1, 512], f32, name="r_sb", tag="r")
                    nc.scalar.activation(r_sb[:], lnw[:], Exp, scale=-1.0)
                    rb_sb = rpool.tile([128, 512], f32, name="rb_sb", tag="rb")
                    nc.gpsimd.partition_broadcast(rb_sb[:], r_sb[:1, :])
                    yt = ypool.tile([128, 2, 512], bf16, name="yt", tag=f"yt{h}")
                    for lt in (0, 1):
                        nc.vector.tensor_mul(yt[:, lt, :], yu[:, lt, :], rb_sb[:])
                    nc.vector.tensor_mul(
                        yt2s[h * 32 : (h + 1) * 32, :], yu2[:], rb_sb[:32]
                    )
                    yts.append(yt)

                    if final and h == HPC - 1:
                        emit_final_back(yt, yt2s, final_osbs, qc)
                if not final:
                    pending.append((yts, yt2s, qc))

    nc.finalize()
    return nc


def _get_nc():
    if "nc" not in _cache:
        _cache["nc"] = _build_nc()
    return _cache["nc"]


def _prep_inputs(x, latent_w, latent_b, Wd_w, Wd_b, out_w):
    """Host-side shard + layout prep. Returns list of 8 per-core input maps."""
    bf16 = ml_dtypes.bfloat16
    x = np.asarray(x, dtype=np.float32)
    latent_w = np.asarray(latent_w, dtype=np.float32)
    latent_b = np.asarray(latent_b, dtype=np.float32)
    Wd_w = np.asarray(Wd_w, dtype=np.float32)
    Wd_b = np.asarray(Wd_b, dtype=np.float32)
    out_w = np.asarray(out_w, dtype=np.float32)

    xT = np.ascontiguousarray(x.transpose(0, 2, 1)).reshape(B, 8, 128, T)
    xT = xT.astype(bf16)

    lw = np.zeros((C, 289), np.float32)
    lw[:, :288] = latent_w
    lw = lw.reshape(8, 128, 289).astype(bf16)

    lbt = np.zeros((128, 3), np.float32)
    for lt, lsz in LT:
        lbt[:lsz, lt] = latent_b[lt * 128 : lt * 128 + lsz]

    # causal masks for the 4 diagonal key tiles: mask[i][tk, tq] = tq >= i*128+tk
    tq = np.arange(512)[None, :]
    tk = np.arange(128)[:, None]
    masks = np.stack([(tq >= i * 128 + tk) for i in range(4)]).astype(np.float32)
    masks = masks.astype(bf16)
    id128 = np.eye(128, dtype=np.float32).astype(bf16)

    # per-head-group weights (shared by the two cores of each group)
    grp_maps = []
    for g in range(CPB):
        heads = [HPC * g + i for i in range(HPC)]
        wd = np.zeros((8, 128, 1152), np.float32)
        wd2 = np.zeros((8, 128, 128), np.float32)
        wdbt = np.zeros((128, 12), np.float32)
        wdbt2 = np.zeros((128, 1), np.float32)
        ow = np.zeros((8, 128, 1024), np.float32)
        ow2 = np.zeros((128, 1024), np.float32)
        for i, h in enumerate(heads):
            ow2[i * 32 : (i + 1) * 32, :] = out_w[h * 288 + 256 : h * 288 + 288, :]
            wd2[:, :, i * 32 : (i + 1) * 32] = (
                Wd_w[h][:, 256:288] / 8.0
            ).reshape(8, 128, 32)
            wdbt2[i * 32 : (i + 1) * 32, 0] = Wd_b[h][256:288] / 8.0
            wd[:, :, i * 288 : (i + 1) * 288] = (Wd_w[h] / 8.0).reshape(8, 128, 288)
            for lt, lsz in LT:
                wdbt[:lsz, i * 3 + lt] = Wd_b[h][lt * 128 : lt * 128 + lsz] / 8.0
                if lt < 2:
                    ow[i * 2 + lt, :lsz, :] = out_w[
                        h * 288 + lt * 128 : h * 288 + lt * 128 + lsz, :
                    ]
        grp_maps.append(
            {
                "wd": wd.astype(bf16),
                "wd2": wd2.astype(bf16),
                "wdbt": wdbt,
                "wdbt2": wdbt2,
                "ow": ow.astype(bf16),
                "ow2": ow2.astype(bf16),
            }
        )

    in_maps = []
    for c in range(NCORES):
        b, g = divmod(c, CPB)
        m = {
            "xT": xT[b],
            "lw": lw,
            "lbt": lbt,
            "masks": masks,
            "id128": id128,
        }
        m.update(grp_maps[g])
        in_maps.append(m)
    return in_maps


def _combine(results, out_b):
    out = np.zeros((B, T, C), np.float64)
    for c in range(NCORES):
        out[c // CPB] += results[c]["out_p"].astype(np.float64)
    out += np.asarray(out_b, dtype=np.float64)[None, None, :]
    return out.astype(np.float32)


def kernel(x, latent_w, latent_b, Wd_w, Wd_b, out_w, out_b, **kw):
    from concourse import bass_utils

    nc = _get_nc()
    in_maps = _prep_inputs(x, latent_w, latent_b, Wd_w, Wd_b, out_w)
    res = bass_utils.run_bass_kernel_spmd(nc, in_maps, core_ids=list(range(NCORES)))
    return _combine(res.results, out_b)


# revision 41
# speedup vs baseline: 1.2590x; 1.0004x over previous
"""Multi-head latent attention (MLA-style) Trainium2 kernel, 8-core SPMD.

Sharding: tensor-parallel over (batch x heads). Core c handles batch
b = c // 4 and the 4 heads 4*(c%4) .. 4*(c%4)+3:
  - kv latent (Wdkv) computed per core for its batch only
  - per-head compressed q, latent-space causal attention, and the head's
    slice of the output projection (row-sharded out_w)
  - per-core output is a PARTIAL [T, C] sum for its batch; host adds the
    4 partials per batch and the output bias.

All matmuls run in bf16 (fp32 PSUM accumulation).

Layouts (host-prepared):
  xT     [8, 128, T]      x[b].T              (c = o*128 + p)
  lw     [8, 128, 289]    latent_w, zero-padded col 288
  lbt    [128, 3]         latent_b per l-tile (fp32)
  wd     [8, 128, 1152]   Wd_w[h]/8 for the core's 4 heads, h*288+l
  wd2    [8, 128, 128]    Wd_w[h][:, 256:288]/8 stacked over 4 heads
  wdbt   [128, 12]        Wd_b[h]/8 per (h, l-tile) (fp32)
  wdbt2  [128, 1]         Wd_b[h][256:288]/8 stacked (fp32)
  ow     [8, 128, 1024]   out_w rows per (h, lt in 0..1)
  ow2    [128, 1024]      out_w l2 rows stacked over 4 heads
  masks  [4, 128, 512]    causal masks for the 4 diagonal key tiles
Output:
  out_p  [2048, 1024] fp32 partial (for the core's batch)
"""

import numpy as np
import ml_dtypes

B, T, C = 2, 2048, 1024
H, L = 16, 288
NCORES = 8
HPC = 4  # heads per core
CPB = NCORES // B  # cores per batch

# l-dimension tiles of L=288 (and the +1 sum row for the y matmul)
LT = [(0, 128), (1, 128), (2, 32)]
MT = [(0, 128), (1, 128), (2, 33)]  # y-matmul M tiles (includes sum row 288)

_cache = {}


def _build_nc():
    import concourse.bacc as bacc
    import concourse.mybir as mybir
    import concourse.tile as tile
    from concourse.bass import ts

    bf16 = mybir.dt.bfloat16
    f32 = mybir.dt.float32

    nc = bacc.Bacc("TRN2", target_bir_lowering=False, debug=True)

    d_xT = nc.dram_tensor("xT", [8, 128, T], bf16, kind="ExternalInput")
    d_lw = nc.dram_tensor("lw", [8, 128, 289], bf16, kind="ExternalInput")
    d_lbt = nc.dram_tensor("lbt", [128, 3], f32, kind="ExternalInput")
    d_wd = nc.dram_tensor("wd", [8, 128, 1152], bf16, kind="ExternalInput")
    d_wd2 = nc.dram_tensor("wd2", [8, 128, 128], bf16, kind="ExternalInput")
    d_wdbt = nc.dram_tensor("wdbt", [128, 12], f32, kind="ExternalInput")
    d_wdbt2 = nc.dram_tensor("wdbt2", [128, 1], f32, kind="ExternalInput")
    d_ow = nc.dram_tensor("ow", [8, 128, 1024], bf16, kind="ExternalInput")
    d_ow2 = nc.dram_tensor("ow2", [128, 1024], bf16, kind="ExternalInput")
    d_masks = nc.dram_tensor("masks", [4, 128, 512], bf16, kind="ExternalInput")
    d_id = nc.dram_tensor("id128", [128, 128], bf16, kind="ExternalInput")
    d_out = nc.dram_tensor("out_p", [T, C], f32, kind="ExternalOutput")

    Exp = mybir.ActivationFunctionType.Exp
    Ident = mybir.ActivationFunctionType.Identity
    Ln = mybir.ActivationFunctionType.Ln

    with tile.TileContext(nc) as tc:
        with (
            tc.tile_pool(name="const", bufs=1) as cpool,
            tc.tile_pool(name="xp", bufs=1) as xpool,
            tc.tile_pool(name="kvp", bufs=1) as kvpool,
            tc.tile_pool(name="qp", bufs=2) as qpool,
            tc.tile_pool(name="ep", bufs=4) as epool,
            tc.tile_pool(name="yp", bufs=2) as ypool,
            tc.tile_pool(name="rp", bufs=2) as rpool,
            tc.tile_pool(name="op", bufs=3) as opool,
            tc.tile_pool(name="ps_y", bufs=1, space="PSUM") as ps_y,
            tc.tile_pool(name="ps_s", bufs=3, space="PSUM") as ps_s,
            tc.tile_pool(name="ps_m", bufs=2, space="PSUM") as ps_m,
        ):
            # ---- persistent weights ----
            # latent_w first: the kvT matmuls only need lw + the first x
            # chunk, so the PE can start ~10us earlier
            lw_sb = cpool.tile([128, 8, 289], bf16, name="lw_sb")
            for kc in range(8):
                # split across the two HWDGE queues to halve the startup
                # serial chain (kv matmuls consume kc in order)
                eng = nc.sync if kc % 2 == 0 else nc.scalar
                eng.dma_start(lw_sb[:, kc, :], d_lw[kc])

            lbt_sb = cpool.tile([128, 3], f32, name="lbt_sb")
            nc.sync.dma_start(lbt_sb[:], d_lbt[:])
            id_sb = cpool.tile([128, 128], bf16, name="id_sb")
            nc.sync.dma_start(id_sb[:], d_id[:])
            wd_sb = cpool.tile([128, 8, 1152], bf16, name="wd_sb")
            wd2_sb = cpool.tile([128, 8, 128], bf16, name="wd2_sb")
            wdbt_sb = cpool.tile([128, 12], f32, name="wdbt_sb")
            wdbt2_sb = cpool.tile([128, 1], f32, name="wdbt2_sb")
            ow_sb = cpool.tile([128, 8, 1024], bf16, name="ow_sb")
            ow2_sb = cpool.tile([128, 1024], bf16, name="ow2_sb")
            masks_sb = cpool.tile([128, 4, 512], bf16, name="masks_sb")

            def load_weights():
                for kc in range(8):
                    nc.sync.dma_start(wd_sb[:, kc, :], d_wd[kc])
                    nc.sync.dma_start(wd2_sb[:, kc, :], d_wd2[kc])
                nc.sync.dma_start(wdbt_sb[:], d_wdbt[:])
                nc.sync.dma_start(wdbt2_sb[:], d_wdbt2[:])
                for i in range(8):
                    nc.sync.dma_start(ow_sb[:, i, :], d_ow[i])
                nc.sync.dma_start(ow2_sb[:], d_ow2[:])
                for i in range(4):
                    nc.sync.dma_start(masks_sb[:, i, :], d_masks[i])

            # deferred out-projection: (yts, qc) emitted one head late so
            # the PE queue never blocks on the normalize chain
            pending = []

            def emit_outproj():
                yts, yt2s, pqc = pending.pop()
                for blk in range(4):
                    osb = opool.tile([128, 1024], f32, name="osb", tag="osb")
                    for cc in range(2):
                        po = ps_m.tile([128, 512], f32, name="ps_o", tag="m")
                        for h in range(HPC):
                            for lt in (0, 1):
                                nc.tensor.matmul(
                                    po,
                                    yts[h][:, lt, ts(blk, 128)],
                                    ow_sb[:, h * 2 + lt, ts(cc, 512)],
                                    start=(h == 0 and lt == 0),
                                    stop=False,
                                )
                        # all 4 heads' l2 blocks stacked into one K=128 matmul
                        nc.tensor.matmul(
                            po,
                            yt2s[:, ts(blk, 128)],
                            ow2_sb[:, ts(cc, 512)],
                            start=False,
                            stop=True,
                        )
                        nc.vector.tensor_copy(osb[:, ts(cc, 512)], po[:])
                    row0 = pqc * 512 + blk * 128
                    nc.sync.dma_start(d_out[row0 : row0 + 128, :], osb[:])

            # the last chunk's out-projection is emitted in two stages
            # (heads 0..2 overlap the last head's attention) to shrink the
            # end-of-kernel tail
            def emit_final_front(yts):
                osbs = []
                for blk in range(4):
                    osb = opool.tile(
                        [128, 1024], f32, name="osbf", tag="osbf", bufs=4
                    )
                    for cc in range(2):
                        po = ps_m.tile([128, 512], f32, name="ps_o", tag="m")
                        for h in range(HPC - 1):
                            for lt in (0, 1):
                                nc.tensor.matmul(
                                    po,
                                    yts[h][:, lt, ts(blk, 128)],
                                    ow_sb[:, h * 2 + lt, ts(cc, 512)],
                                    start=(h == 0 and lt == 0),
                                    stop=(h == HPC - 2 and lt == 1),
                                )
                        nc.vector.tensor_copy(osb[:, ts(cc, 512)], po[:])
                    osbs.append(osb)
                return osbs

            def emit_final_back(yt, yt2f, osbs, pqc):
                hl = HPC - 1
                for blk in range(4):
                    for cc in range(2):
                        po = ps_m.tile([128, 512], f32, name="ps_o", tag="m")
                        for lt in (0, 1):
                            nc.tensor.matmul(
                                po,
                                yt[:, lt, ts(blk, 128)],
                                ow_sb[:, hl * 2 + lt, ts(cc, 512)],
                                start=(lt == 0),
                                stop=False,
                            )
                        nc.tensor.matmul(
                            po,
                            yt2f[:, ts(blk, 128)],
                            ow2_sb[:, ts(cc, 512)],
                            start=False,
                            stop=True,
                        )
                        nc.vector.tensor_add(
                            osbs[blk][:, ts(cc, 512)],
                            po[:],
                            osbs[blk][:, ts(cc, 512)],
                        )
                    row0 = pqc * 512 + blk * 128
                    nc.sync.dma_start(d_out[row0 : row0 + 128, :], osbs[blk][:])

            # ---- load x^T, per 512-chunk (SWDGE queues, parallel to the
            # HWDGE weight loads) ----
            xts = []
            for tch in range(4):
                xt = xpool.tile([128, 8, 512], bf16, name="xt", tag=f"xT{tch}")
                for o in range(8):
                    nc.gpsimd.dma_start(xt[:, o, :], d_xT[o][:, ts(tch, 512)])
                xts.append(xt)
            load_weights()

            # ---- kvT = (x @ latent_w + latent_b)^T : [l, t], per chunk;
            #      kv_aug[t, 0:289] = [kv | 1] via PE transpose ----
            def compute_kv(xtile, ktag):
                kvt = kvpool.tile([128, 3, 512], bf16, name="kvt", tag=f"kvT{ktag}")
                for lt, lsz in LT:
                    pq = ps_s.tile([128, 512], f32, name="ps_kv", tag="s")
                    for kc in range(8):
                        nc.tensor.matmul(
                            pq[:lsz],
                            lw_sb[:, kc, lt * 128 : lt * 128 + lsz],
                            xtile[:, kc, :],
                            start=(kc == 0),
                            stop=(kc == 7),
                        )
                    nc.scalar.activation(
                        kvt[:lsz, lt, :],
                        pq[:lsz],
                        Ident,
                        bias=lbt_sb[:lsz, lt : lt + 1],
                    )

                # kv-l2 relaid out so adjacent t-tiles sit at partition
                # offsets 0/32, enabling paired (concurrent) K=32 matmuls
                kv2p = kvpool.tile([64, 2, 128], bf16, name="kv2p", tag=f"kv2p{ktag}")
                for j in range(4):
                    nc.sync.dma_start(
                        kv2p[32 * (j % 2) : 32 * (j % 2) + 32, j // 2, :],
                        kvt[:32, 2, ts(j, 128)],
                    )

                kva = kvpool.tile([128, 4, 289], bf16, name="kva", tag=f"kva{ktag}")
                for tt in range(4):
                    nc.vector.memset(kva[:, tt, 288:289], 1.0)
                    for lt, lsz in LT:
                        pt = ps_m.tile([128, 512], bf16, name="ps_t", tag="m")
                        nc.tensor.transpose(
                            pt[:, :lsz],
                            kvt[:lsz, lt, ts(tt, 128)],
                            id_sb[:lsz, :lsz],
                        )
                        nc.vector.tensor_copy(
                            kva[:, tt, lt * 128 : lt * 128 + lsz], pt[:, :lsz]
                        )
                return kvt, kv2p, kva

            kvts, kv2ps, kvas = [], [], []
            for tch in range(4):
                kvt, kv2p, kva = compute_kv(xts[tch], tch)
                kvts.append(kvt)
                kv2ps.append(kv2p)
                kvas.append(kva)

            # ---- attention per (chunk, head) ----
            for qc in range(4):
                final = qc == 3
                yts = []
                yt2s = ypool.tile([128, 512], bf16, name="yt2s", tag="yt2")

                # all 4 heads' l2 (l=256..287) q-projection stacked into
                # one M=128 matmul group; each head's half is then
                # DMA-replicated at partition offsets 0/32 so the paired
                # scores matmul K ranges line up
                pq2 = ps_s.tile([128, 512], f32, name="ps_q2", tag="s")
                for kc in range(8):
                    nc.tensor.matmul(
                        pq2,
                        wd2_sb[:, kc, :],
                        xts[qc][:, kc, :],
                        start=(kc == 0),
                        stop=(kc == 7),
                    )
                qt2w = qpool.tile([128, 512], bf16, name="qt2w", tag="qt2w")
                nc.scalar.activation(
                    qt2w[:], pq2[:], Ident, bias=wdbt2_sb[:, 0:1]
                )
                # per-head pair-replica: qrep[0:32,h]=qrep[32:64,h]=q2_h
                qrep = qpool.tile([64, 4, 512], bf16, name="qrep", tag="qrep")
                for h in range(HPC):
                    nc.sync.dma_start(qrep[0:32, h, :], qt2w[32 * h : 32 * h + 32, :])
                    nc.gpsimd.dma_start(
                        qrep[32:64, h, :], qt2w[32 * h : 32 * h + 32, :]
                    )

                for h in range(HPC):
                    # q^T chunk [l, 512] (scale 1/8 folded into wd)
                    qt = qpool.tile([128, 2, 512], bf16, name="qt", tag="qt")
                    for lt in (0, 1):
                        pq = ps_s.tile([128, 512], f32, name="ps_q", tag="s")
                        for kc in range(8):
                            nc.tensor.matmul(
                                pq,
                                wd_sb[:, kc, h * 288 + lt * 128 :][:, :128],
                                xts[qc][:, kc, :],
                                start=(kc == 0),
                                stop=(kc == 7),
                            )
                        nc.scalar.activation(
                            qt[:, lt, :],
                            pq[:],
                            Ident,
                            bias=wdbt_sb[:, h * 3 + lt : h * 3 + lt + 1],
                        )

                    # scores^T -> exp -> (mask) -> y accumulation
                    py = [
                        ps_y.tile([128, 512], f32, name=f"ps_y{mt}", tag=f"y{mt}")
                        for mt, _ in MT
                    ]
                    ntk = qc * 4 + 4

                    def emit_y(tk, et, c0):
                        for mt, msz in MT:
                            nc.tensor.matmul(
                                py[mt][:msz, c0:],
                                kvas[tk // 4][:, tk % 4, mt * 128 :][:, :msz],
                                et[:, c0:],
                                start=(tk == 0),
                                stop=(tk == ntk - 1),
                            )

                    # scores/exp pipelined one pair ahead of the y matmuls
                    # so the PE queue never blocks on the ACT exp; the two
                    # K=32 l2 matmuls of each pair run in concurrent PE
                    # row groups (partition offsets 0 / 32)
                    pend = []
                    for pr in range(ntk // 2):
                        pair = []
                        for tk in (2 * pr, 2 * pr + 1):
                            # diagonal tiles: only columns >= c0 unmasked
                            c0 = max(0, (tk - qc * 4) * 128)
                            pss = ps_s.tile(
                                [128, 512], f32, name="ps_s", tag="s"
                            )
                            for lt in (0, 1):
                                nc.tensor.matmul(
                                    pss[:, c0:],
                                    kvts[tk // 4][:, lt, ts(tk % 4, 128)],
                                    qt[:, lt, c0:],
                                    start=(lt == 0),
                                    stop=False,
                                )
                            pair.append((tk, pss, c0))
                        for off, (tk, pss, c0) in zip((0, 32), pair):
                            nc.tensor.matmul(
                                pss[:, c0:],
                                kv2ps[tk // 4][
                                    off : off + 32, (tk % 4) // 2, :
                                ],
                                qrep[off : off + 32, h, c0:],
                                start=False,
                                stop=True,
                            )
                        for tk, pss, c0 in pair:
                            et = epool.tile(
                                [128, 512], bf16, name="et", tag="et"
                            )
                            nc.scalar.activation(et[:, c0:], pss[:, c0:], Exp)
                            i = tk - qc * 4
                            if i >= 0:
                                # mask is nontrivial only in the i-th
                                # 128-column block
                                nc.vector.tensor_mul(
                                    et[:, c0 : c0 + 128],
                                    et[:, c0 : c0 + 128],
                                    masks_sb[:, i, c0 : c0 + 128],
                                )
                            pend.append((tk, et, c0))
                        while len(pend) > 2:
                            emit_y(*pend.pop(0))
                        if final and h == HPC - 1 and pr == 3:
                            # earlier heads' deferred out-projection, emitted
                            # here so its matmuls enter the PE queue well
                            # after their normalize chains have completed
                            final_osbs = emit_final_front(yts)
                    for e in pend:
                        emit_y(*e)

                    # drain the PSUM banks immediately (unnormalized), so the
                    # next head's matmuls never wait on the normalize chain
                    lnw = rpool.tile([1, 512], f32, name="lnw", tag="lnw")
                    nc.scalar.activation(lnw[:], py[2][32:33, :], Ln)
                    yu = ypool.tile(
                        [128, 2, 512], bf16, name="yu", tag=f"yu{h}", bufs=1
                    )
                    for lt in (0, 1):
                        nc.vector.tensor_copy(yu[:, lt, :], py[lt][:])
                    yu2 = rpool.tile([32, 512], bf16, name="yu2", tag=f"yu2{h}")
                    nc.vector.tensor_copy(yu2[:], py[2][:32])

                    # prev-head out-projection enqueues (PE + DVE copies)
                    # ahead of the normalize tail in the engine FIFOs
                    if pending:
                        emit_outproj()

                    # r = exp(-ln(sum)) = 1/sum, entirely on the scalar
                    # engine: keeps the serial 3.3us DVE reciprocal out of
                    # the DVE FIFO that the yt muls (and thus the deferred
                    # out-projection) queue behind
                    r_sb = rpool.tile([1, 512], f32, name="r_sb", tag="r")
                    nc.scalar.activation(r_sb[:], lnw[:], Exp, scale=-1.0)
                    rb_sb = rpool.tile([128, 512], f32, name="rb_sb", tag="rb")
                    nc.gpsimd.partition_broadcast(rb_sb[:], r_sb[:1, :])
                    yt = ypool.tile([128, 2, 512], bf16, name="yt", tag=f"yt{h}")
                    for lt in (0, 1):
                        nc.vector.tensor_mul(yt[:, lt, :], yu[:, lt, :], rb_sb[:])
                    nc.vector.tensor_mul(
                        yt2s[h * 32 : (h + 1) * 32, :], yu2[:], rb_sb[:32]
                    )
                    yts.append(yt)

                    if final and h == HPC - 1:
                        emit_final_back(yt, yt2s, final_osbs, qc)
                if not final:
                    pending.append((yts, yt2s, qc))

    nc.finalize()
    return nc


def _get_nc():
    if "nc" not in _cache:
        _cache["nc"] = _build_nc()
    return _cache["nc"]


def _prep_inputs(x, latent_w, latent_b, Wd_w, Wd_b, out_w):
    """Host-side shard + layout prep. Returns list of 8 per-core input maps."""
    bf16 = ml_dtypes.bfloat16
    x = np.asarray(x, dtype=np.float32)
    latent_w = np.asarray(latent_w, dtype=np.float32)
    latent_b = np.asarray(latent_b, dtype=np.float32)
    Wd_w = np.asarray(Wd_w, dtype=np.float32)
    Wd_b = np.asarray(Wd_b, dtype=np.float32)
    out_w = np.asarray(out_w, dtype=np.float32)

    xT = np.ascontiguousarray(x.transpose(0, 2, 1)).reshape(B, 8, 128, T)
    xT = xT.astype(bf16)

    lw = np.zeros((C, 289), np.float32)
    lw[:, :288] = latent_w
    lw = lw.reshape(8, 128, 289).astype(bf16)

    lbt = np.zeros((128, 3), np.float32)
    for lt, lsz in LT:
        lbt[:lsz, lt] = latent_b[lt * 128 : lt * 128 + lsz]

    # causal masks for the 4 diagonal key tiles: mask[i][tk, tq] = tq >= i*128+tk
    tq = np.arange(512)[None, :]
    tk = np.arange(128)[:, None]
    masks = np.stack([(tq >= i * 128 + tk) for i in range(4)]).astype(np.float32)
    masks = masks.astype(bf16)
    id128 = np.eye(128, dtype=np.float32).astype(bf16)

    # per-head-group weights (shared by the two cores of each group)
    grp_maps = []
    for g in range(CPB):
        heads = [HPC * g + i for i in range(HPC)]
        wd = np.zeros((8, 128, 1152), np.float32)
        wd2 = np.zeros((8, 128, 128), np.float32)
        wdbt = np.zeros((128, 12), np.float32)
        wdbt2 = np.zeros((128, 1), np.float32)
        ow = np.zeros((8, 128, 1024), np.float32)
        ow2 = np.zeros((128, 1024), np.float32)
        for i, h in enumerate(heads):
            ow2[i * 32 : (i + 1) * 32, :] = out_w[h * 288 + 256 : h * 288 + 288, :]
            wd2[:, :, i * 32 : (i + 1) * 32] = (
                Wd_w[h][:, 256:288] / 8.0
            ).reshape(8, 128, 32)
            wdbt2[i * 32 : (i + 1) * 32, 0] = Wd_b[h][256:288] / 8.0
            wd[:, :, i * 288 : (i + 1) * 288] = (Wd_w[h] / 8.0).reshape(8, 128, 288)
            for lt, lsz in LT:
                wdbt[:lsz, i * 3 + lt] = Wd_b[h][lt * 128 : lt * 128 + lsz] / 8.0
                if lt < 2:
                    ow[i * 2 + lt, :lsz, :] = out_w[
                        h * 288 + lt * 128 : h * 288 + lt * 128 + lsz, :
                    ]
        grp_maps.append(
            {
                "wd": wd.astype(bf16),
                "wd2": wd2.astype(bf16),
                "wdbt": wdbt,
                "wdbt2": wdbt2,
                "ow": ow.astype(bf16),
                "ow2": ow2.astype(bf16),
            }
        )

    in_maps = []
    for c in range(NCORES):
        b, g = divmod(c, CPB)
        m = {
            "xT": xT[b],
            "lw": lw,
            "lbt": lbt,
            "masks": masks,
            "id128": id128,
        }
        m.update(grp_maps[g])
        in_maps.append(m)
    return in_maps


def _combine(results, out_b):
    out = np.zeros((B, T, C), np.float64)
    for c in range(NCORES):
        out[c // CPB] += results[c]["out_p"].astype(np.float64)
    out += np.asarray(out_b, dtype=np.float64)[None, None, :]
    return out.astype(np.float32)


def kernel(x, latent_w, latent_b, Wd_w, Wd_b, out_w, out_b, **kw):
    from concourse import bass_utils

    nc = _get_nc()
    in_maps = _prep_inputs(x, latent_w, latent_b, Wd_w, Wd_b, out_w)
    res = bass_utils.run_bass_kernel_spmd(nc, in_maps, core_ids=list(range(NCORES)))
    return _combine(res.results, out_b)


# revision 45
# speedup vs baseline: 1.2656x; 1.0053x over previous
"""Multi-head latent attention (MLA-style) Trainium2 kernel, 8-core SPMD.

Sharding: tensor-parallel over (batch x heads). Core c handles batch
b = c // 4 and the 4 heads 4*(c%4) .. 4*(c%4)+3:
  - kv latent (Wdkv) computed per core for its batch only
  - per-head compressed q, latent-space causal attention, and the head's
    slice of the output projection (row-sharded out_w)
  - per-core output is a PARTIAL [T, C] sum for its batch; host adds the
    4 partials per batch and the output bias.

All matmuls run in bf16 (fp32 PSUM accumulation).

Layouts (host-prepared):
  xT     [8, 128, T]      x[b].T              (c = o*128 + p)
  lw     [8, 128, 289]    latent_w, zero-padded col 288
  lbt    [128, 3]         latent_b per l-tile (fp32)
  wd     [8, 128, 1152]   Wd_w[h]/8 for the core's 4 heads, h*288+l
  wd2    [8, 128, 128]    Wd_w[h][:, 256:288]/8 stacked over 4 heads
  wdbt   [128, 12]        Wd_b[h]/8 per (h, l-tile) (fp32)
  wdbt2  [128, 1]         Wd_b[h][256:288]/8 stacked (fp32)
  ow     [8, 128, 1024]   out_w rows per (h, lt in 0..1)
  ow2    [128, 1024]      out_w l2 rows stacked over 4 heads
  masks  [4, 128, 512]    causal masks for the 4 diagonal key tiles
Output:
  out_p  [2048, 1024] bf16 partial (for the core's batch)
"""

import numpy as np
import ml_dtypes

B, T, C = 2, 2048, 1024
H, L = 16, 288
NCORES = 8
HPC = 4  # heads per core
CPB = NCORES // B  # cores per batch

# l-dimension tiles of L=288 (and the +1 sum row for the y matmul)
LT = [(0, 128), (1, 128), (2, 32)]
MT = [(0, 128), (1, 128), (2, 33)]  # y-matmul M tiles (includes sum row 288)

_cache = {}


def _build_nc():
    import concourse.bacc as bacc
    import concourse.mybir as mybir
    import concourse.tile as tile
    from concourse.bass import ts

    bf16 = mybir.dt.bfloat16
    f32 = mybir.dt.float32

    nc = bacc.Bacc("TRN2", target_bir_lowering=False, debug=True)

    d_xT = nc.dram_tensor("xT", [8, 128, T], bf16, kind="ExternalInput")
    d_lw = nc.dram_tensor("lw", [8, 128, 289], bf16, kind="ExternalInput")
    d_lbt = nc.dram_tensor("lbt", [128, 3], f32, kind="ExternalInput")
    d_wd = nc.dram_tensor("wd", [8, 128, 1152], bf16, kind="ExternalInput")
    d_wd2 = nc.dram_tensor("wd2", [8, 128, 128], bf16, kind="ExternalInput")
    d_wdbt = nc.dram_tensor("wdbt", [128, 12], f32, kind="ExternalInput")
    d_wdbt2 = nc.dram_tensor("wdbt2", [128, 1], f32, kind="ExternalInput")
    d_ow = nc.dram_tensor("ow", [8, 128, 1024], bf16, kind="ExternalInput")
    d_ow2 = nc.dram_tensor("ow2", [128, 1024], bf16, kind="ExternalInput")
    d_masks = nc.dram_tensor("masks", [4, 128, 512], bf16, kind="ExternalInput")
    d_id = nc.dram_tensor("id128", [128, 128], bf16, kind="ExternalInput")
    d_out = nc.dram_tensor("out_p", [T, C], bf16, kind="ExternalOutput")

    Exp = mybir.ActivationFunctionType.Exp
    Ident = mybir.ActivationFunctionType.Identity
    Ln = mybir.ActivationFunctionType.Ln

    with tile.TileContext(nc) as tc:
        with (
            tc.tile_pool(name="const", bufs=1) as cpool,
            tc.tile_pool(name="xp", bufs=1) as xpool,
            tc.tile_pool(name="kvp", bufs=1) as kvpool,
            tc.tile_pool(name="qp", bufs=2) as qpool,
            tc.tile_pool(name="ep", bufs=4) as epool,
            tc.tile_pool(name="yp", bufs=2) as ypool,
            tc.tile_pool(name="rp", bufs=2) as rpool,
            tc.tile_pool(name="op", bufs=3) as opool,
            tc.tile_pool(name="ps_y", bufs=1, space="PSUM") as ps_y,
            tc.tile_pool(name="ps_s", bufs=3, space="PSUM") as ps_s,
            tc.tile_pool(name="ps_m", bufs=2, space="PSUM") as ps_m,
            tc.tile_pool(name="dram", bufs=2, space="DRAM") as dram,
        ):
            # ---- persistent weights ----
            # latent_w first: the kvT matmuls only need lw + the first x
            # chunk, so the PE can start ~10us earlier
            lw_sb = cpool.tile([128, 8, 289], bf16, name="lw_sb")
            for kc in range(8):
                # split across the two HWDGE queues to halve the startup
                # serial chain (kv matmuls consume kc in order)
                eng = nc.sync if kc % 2 == 0 else nc.scalar
                eng.dma_start(lw_sb[:, kc, :], d_lw[kc])

            lbt_sb = cpool.tile([128, 3], f32, name="lbt_sb")
            nc.sync.dma_start(lbt_sb[:], d_lbt[:])
            id_sb = cpool.tile([128, 128], bf16, name="id_sb")
            nc.sync.dma_start(id_sb[:], d_id[:])
            wd_sb = cpool.tile([128, 8, 1152], bf16, name="wd_sb")
            wd2_sb = cpool.tile([128, 8, 128], bf16, name="wd2_sb")
            wdbt_sb = cpool.tile([128, 12], f32, name="wdbt_sb")
            wdbt2_sb = cpool.tile([128, 1], f32, name="wdbt2_sb")
            ow_sb = cpool.tile([128, 8, 1024], bf16, name="ow_sb")
            ow2_sb = cpool.tile([128, 1024], bf16, name="ow2_sb")
            masks_sb = cpool.tile([128, 4, 512], bf16, name="masks_sb")

            def load_weights():
                for kc in range(8):
                    nc.sync.dma_start(wd_sb[:, kc, :], d_wd[kc])
                    nc.sync.dma_start(wd2_sb[:, kc, :], d_wd2[kc])
                nc.sync.dma_start(wdbt_sb[:], d_wdbt[:])
                nc.sync.dma_start(wdbt2_sb[:], d_wdbt2[:])
                for i in range(8):
                    nc.sync.dma_start(ow_sb[:, i, :], d_ow[i])
                nc.sync.dma_start(ow2_sb[:], d_ow2[:])
                for i in range(4):
                    nc.sync.dma_start(masks_sb[:, i, :], d_masks[i])

            # deferred out-projection: (yts, qc) emitted one head late so
            # the PE queue never blocks on the normalize chain
            pending = []

            def emit_outproj():
                yts, yt2s, pqc = pending.pop()
                for blk in range(4):
                    osb = opool.tile([128, 1024], bf16, name="osb", tag="osb")
                    for cc in range(2):
                        po = ps_m.tile([128, 512], f32, name="ps_o", tag="m")
                        for h in range(HPC):
                            for lt in (0, 1):
                                nc.tensor.matmul(
                                    po,
                                    yts[h][:, lt, ts(blk, 128)],
                                    ow_sb[:, h * 2 + lt, ts(cc, 512)],
                                    start=(h == 0 and lt == 0),
                                    stop=False,
                                )
                        # all 4 heads' l2 blocks stacked into one K=128 matmul
                        nc.tensor.matmul(
                            po,
                            yt2s[:, ts(blk, 128)],
                            ow2_sb[:, ts(cc, 512)],
                            start=False,
                            stop=True,
                        )
                        nc.vector.tensor_copy(osb[:, ts(cc, 512)], po[:])
                    row0 = pqc * 512 + blk * 128
                    nc.sync.dma_start(d_out[row0 : row0 + 128, :], osb[:])

            # the last chunk's out-projection is emitted in two stages
            # (heads 0..2 overlap the last head's attention) to shrink the
            # end-of-kernel tail
            def emit_final_front(yts):
                osbs = []
                for blk in range(4):
                    osb = opool.tile(
                        [128, 1024], bf16, name="osbf", tag="osbf", bufs=4
                    )
                    for cc in range(2):
                        po = ps_m.tile([128, 512], f32, name="ps_o", tag="m")
                        for h in range(HPC - 1):
                            for lt in (0, 1):
                                nc.tensor.matmul(
                                    po,
                                    yts[h][:, lt, ts(blk, 128)],
                                    ow_sb[:, h * 2 + lt, ts(cc, 512)],
                                    start=(h == 0 and lt == 0),
                                    stop=(h == HPC - 2 and lt == 1),
                                )
                        nc.vector.tensor_copy(osb[:, ts(cc, 512)], po[:])
                    osbs.append(osb)
                return osbs

            def emit_final_back(yt, yt2f, osbs, pqc):
                hl = HPC - 1
                for blk in range(4):
                    for cc in range(2):
                        po = ps_m.tile([128, 512], f32, name="ps_o", tag="m")
                        for lt in (0, 1):
                            nc.tensor.matmul(
                                po,
                                yt[:, lt, ts(blk, 128)],
                                ow_sb[:, hl * 2 + lt, ts(cc, 512)],
                                start=(lt == 0),
                                stop=False,
                            )
                        nc.tensor.matmul(
                            po,
                            yt2f[:, ts(blk, 128)],
                            ow2_sb[:, ts(cc, 512)],
                            start=False,
                            stop=True,
                        )
                        nc.vector.tensor_add(
                            osbs[blk][:, ts(cc, 512)],
                            po[:],
                            osbs[blk][:, ts(cc, 512)],
                        )
                    row0 = pqc * 512 + blk * 128
                    nc.sync.dma_start(d_out[row0 : row0 + 128, :], osbs[blk][:])

            # ---- load x^T, per 512-chunk (SWDGE queues, parallel to the
            # HWDGE weight loads) ----
            xts = []
            for tch in range(4):
                xt = xpool.tile([128, 8, 512], bf16, name="xt", tag=f"xT{tch}")
                for o in range(8):
                    nc.gpsimd.dma_start(xt[:, o, :], d_xT[o][:, ts(tch, 512)])
                xts.append(xt)
            load_weights()

            # ---- kvT = (x @ latent_w + latent_b)^T : [l, t], per chunk;
            #      kv_aug[t, 0:289] = [kv | 1] via PE transpose ----
            def compute_kv(xtile, ktag):
                kvt = kvpool.tile([128, 3, 512], bf16, name="kvt", tag=f"kvT{ktag}")
                for lt, lsz in LT:
                    pq = ps_s.tile([128, 512], f32, name="ps_kv", tag="s")
                    for kc in range(8):
                        nc.tensor.matmul(
                            pq[:lsz],
                            lw_sb[:, kc, lt * 128 : lt * 128 + lsz],
                            xtile[:, kc, :],
                            start=(kc == 0),
                            stop=(kc == 7),
                        )
                    nc.scalar.activation(
                        kvt[:lsz, lt, :],
                        pq[:lsz],
                        Ident,
                        bias=lbt_sb[:lsz, lt : lt + 1],
                    )

                # kv-l2 relaid out so adjacent t-tiles sit at partition
                # offsets 0/32, enabling paired (concurrent) K=32 matmuls
                kv2p = kvpool.tile([64, 2, 128], bf16, name="kv2p", tag=f"kv2p{ktag}")
                for j in range(4):
                    nc.sync.dma_start(
                        kv2p[32 * (j % 2) : 32 * (j % 2) + 32, j // 2, :],
                        kvt[:32, 2, ts(j, 128)],
                    )

                kva = kvpool.tile([128, 4, 289], bf16, name="kva", tag=f"kva{ktag}")
                for tt in range(4):
                    nc.vector.memset(kva[:, tt, 288:289], 1.0)
                    for lt, lsz in LT:
                        pt = ps_m.tile([128, 512], bf16, name="ps_t", tag="m")
                        nc.tensor.transpose(
                            pt[:, :lsz],
                            kvt[:lsz, lt, ts(tt, 128)],
                            id_sb[:lsz, :lsz],
                        )
                        nc.vector.tensor_copy(
                            kva[:, tt, lt * 128 : lt * 128 + lsz], pt[:, :lsz]
                        )
                return kvt, kv2p, kva

            kvts, kv2ps, kvas = [], [], []
            for tch in range(4):
                kvt, kv2p, kva = compute_kv(xts[tch], tch)
                kvts.append(kvt)
                kv2ps.append(kv2p)
                kvas.append(kva)

            # ---- attention per (chunk, head) ----
            for qc in range(4):
                final = qc == 3
                yts = []
                yt2s = ypool.tile([128, 512], bf16, name="yt2s", tag="yt2")

                # all 4 heads' l2 (l=256..287) q-projection stacked into
                # one M=128 matmul group; each head's half is then
                # DMA-replicated at partition offsets 0/32 so the paired
                # scores matmul K ranges line up
                pq2 = ps_s.tile([128, 512], f32, name="ps_q2", tag="s")
                for kc in range(8):
                    nc.tensor.matmul(
                        pq2,
                        wd2_sb[:, kc, :],
                        xts[qc][:, kc, :],
                        start=(kc == 0),
                        stop=(kc == 7),
                    )
                qt2w = qpool.tile([128, 512], bf16, name="qt2w", tag="qt2w")
                nc.scalar.activation(
                    qt2w[:], pq2[:], Ident, bias=wdbt2_sb[:, 0:1]
                )
                # per-head pair-replica: qrep[0:32,h]=qrep[32:64,h]=q2_h
                qrep = qpool.tile([64, 4, 512], bf16, name="qrep", tag="qrep")
                for h in range(HPC):
                    nc.sync.dma_start(qrep[0:32, h, :], qt2w[32 * h : 32 * h + 32, :])
                    nc.gpsimd.dma_start(
                        qrep[32:64, h, :], qt2w[32 * h : 32 * h + 32, :]
                    )

                for h in range(HPC):
                    # q^T chunk [l, 512] (scale 1/8 folded into wd)
                    qt = qpool.tile([128, 2, 512], bf16, name="qt", tag="qt")
                    for lt in (0, 1):
                        pq = ps_s.tile([128, 512], f32, name="ps_q", tag="s")
                        for kc in range(8):
                            nc.tensor.matmul(
                                pq,
                                wd_sb[:, kc, h * 288 + lt * 128 :][:, :128],
                                xts[qc][:, kc, :],
                                start=(kc == 0),
                                stop=(kc == 7),
                            )
                        nc.scalar.activation(
                            qt[:, lt, :],
                            pq[:],
                            Ident,
                            bias=wdbt_sb[:, h * 3 + lt : h * 3 + lt + 1],
                        )

                    # scores^T -> exp -> (mask) -> y accumulation
                    py = [
                        ps_y.tile([128, 512], f32, name=f"ps_y{mt}", tag=f"y{mt}")
                        for mt, _ in MT
                    ]
                    ntk = qc * 4 + 4

                    def emit_y(tk, et, c0):
                        for mt, msz in MT:
                            nc.tensor.matmul(
                                py[mt][:msz, c0:],
                                kvas[tk // 4][:, tk % 4, mt * 128 :][:, :msz],
                                et[:, c0:],
                                start=(tk == 0),
                                stop=(tk == ntk - 1),
                            )

                    # scores/exp pipelined one pair ahead of the y matmuls
                    # so the PE queue never blocks on the ACT exp; the two
                    # K=32 l2 matmuls of each pair run in concurrent PE
                    # row groups (partition offsets 0 / 32)
                    pend = []
                    for pr in range(ntk // 2):
                        pair = []
                        for tk in (2 * pr, 2 * pr + 1):
                            # diagonal tiles: only columns >= c0 unmasked
                            c0 = max(0, (tk - qc * 4) * 128)
                            pss = ps_s.tile(
                                [128, 512], f32, name="ps_s", tag="s"
                            )
                            for lt in (0, 1):
                                nc.tensor.matmul(
                                    pss[:, c0:],
                                    kvts[tk // 4][:, lt, ts(tk % 4, 128)],
                                    qt[:, lt, c0:],
                                    start=(lt == 0),
                                    stop=False,
                                )
                            pair.append((tk, pss, c0))
                        for off, (tk, pss, c0) in zip((0, 32), pair):
                            nc.tensor.matmul(
                                pss[:, c0:],
                                kv2ps[tk // 4][
                                    off : off + 32, (tk % 4) // 2, :
                                ],
                                qrep[off : off + 32, h, c0:],
                                start=False,
                                stop=True,
                            )
                        for tk, pss, c0 in pair:
                            et = epool.tile(
                                [128, 512], bf16, name="et", tag="et"
                            )
                            nc.scalar.activation(et[:, c0:], pss[:, c0:], Exp)
                            i = tk - qc * 4
                            if i >= 0:
                                # mask is nontrivial only in the i-th
                                # 128-column block
                                nc.vector.tensor_mul(
                                    et[:, c0 : c0 + 128],
                                    et[:, c0 : c0 + 128],
                                    masks_sb[:, i, c0 : c0 + 128],
                                )
                            pend.append((tk, et, c0))
                        while len(pend) > 2:
                            emit_y(*pend.pop(0))
                        if final and h == HPC - 1 and pr == 3:
                            # earlier heads' deferred out-projection, emitted
                            # here so its matmuls enter the PE queue well
                            # after their normalize chains have completed
                            final_osbs = emit_final_front(yts)
                    for e in pend:
                        emit_y(*e)

                    # drain the PSUM banks immediately (unnormalized), so the
                    # next head's matmuls never wait on the normalize chain;
                    # Identity doesn't touch the ACT function table
                    lnw = rpool.tile([1, 512], f32, name="lnw", tag="lnw")
                    nc.scalar.activation(lnw[:], py[2][32:33, :], Ident)
                    yu = ypool.tile(
                        [128, 2, 512], bf16, name="yu", tag=f"yu{h}", bufs=1
                    )
                    for lt in (0, 1):
                        nc.vector.tensor_copy(yu[:, lt, :], py[lt][:])
                    yu2 = rpool.tile([32, 512], bf16, name="yu2", tag=f"yu2{h}")
                    nc.vector.tensor_copy(yu2[:], py[2][:32])

                    # prev-head out-projection enqueues (PE + DVE copies)
                    # ahead of the normalize tail in the engine FIFOs
                    if pending:
                        emit_outproj()

                    # r = 1/sum via DVE reciprocal on a [32,16] layout
                    # (per-lane cost 16 elements instead of 512, ~100ns vs
                    # 3.3us, so it never clogs the DVE FIFO). The reshape
                    # goes through a DRAM bounce with plain linear DMAs —
                    # SBUF->SBUF reshape APs are not expressible (partition
                    # dim is physical). This also keeps Ln off the scalar
                    # engine, whose Exp<->Ln function-table reloads cost
                    # 1.3us twice per head.
                    d_ss = dram.tile([32, 16], f32, name="d_ss", tag="d_ss")
                    nc.sync.dma_start(d_ss[:], lnw[:1, :])
                    s4 = rpool.tile([32, 16], f32, name="s4", tag="s4")
                    nc.sync.dma_start(s4[:], d_ss[:])
                    r4 = rpool.tile([32, 16], f32, name="r4", tag="r4")
                    nc.vector.reciprocal(r4[:], s4[:])
                    d_r = dram.tile([32, 16], f32, name="d_r", tag="d_r")
                    nc.scalar.dma_start(d_r[:], r4[:])
                    r_sb = rpool.tile([1, 512], f32, name="r_sb", tag="r")
                    nc.scalar.dma_start(r_sb[:1, :], d_r[:])
                    rb_sb = rpool.tile([128, 512], f32, name="rb_sb", tag="rb")
                    nc.gpsimd.partition_broadcast(rb_sb[:], r_sb[:1, :])
                    yt = ypool.tile([128, 2, 512], bf16, name="yt", tag=f"yt{h}")
                    for lt in (0, 1):
                        nc.vector.tensor_mul(yt[:, lt, :], yu[:, lt, :], rb_sb[:])
                    nc.vector.tensor_mul(
                        yt2s[h * 32 : (h + 1) * 32, :], yu2[:], rb_sb[:32]
                    )
                    yts.append(yt)

                    if final and h == HPC - 1:
                        emit_final_back(yt, yt2s, final_osbs, qc)
                if not final:
                    pending.append((yts, yt2s, qc))

    nc.finalize()
    return nc


def _get_nc():
    if "nc" not in _cache:
        _cache["nc"] = _build_nc()
    return _cache["nc"]


def _prep_inputs(x, latent_w, latent_b, Wd_w, Wd_b, out_w):
    """Host-side shard + layout prep. Returns list of 8 per-core input maps."""
    bf16 = ml_dtypes.bfloat16
    x = np.asarray(x, dtype=np.float32)
    latent_w = np.asarray(latent_w, dtype=np.float32)
    latent_b = np.asarray(latent_b, dtype=np.float32)
    Wd_w = np.asarray(Wd_w, dtype=np.float32)
    Wd_b = np.asarray(Wd_b, dtype=np.float32)
    out_w = np.asarray(out_w, dtype=np.float32)

    xT = np.ascontiguousarray(x.transpose(0, 2, 1)).reshape(B, 8, 128, T)
    xT = xT.astype(bf16)

    lw = np.zeros((C, 289), np.float32)
    lw[:, :288] = latent_w
    lw = lw.reshape(8, 128, 289).astype(bf16)

    lbt = np.zeros((128, 3), np.float32)
    for lt, lsz in LT:
        lbt[:lsz, lt] = latent_b[lt * 128 : lt * 128 + lsz]

    # causal masks for the 4 diagonal key tiles: mask[i][tk, tq] = tq >= i*128+tk
    tq = np.arange(512)[None, :]
    tk = np.arange(128)[:, None]
    masks = np.stack([(tq >= i * 128 + tk) for i in range(4)]).astype(np.float32)
    masks = masks.astype(bf16)
    id128 = np.eye(128, dtype=np.float32).astype(bf16)

    # per-head-group weights (shared by the two cores of each group)
    grp_maps = []
    for g in range(CPB):
        heads = [HPC * g + i for i in range(HPC)]
        wd = np.zeros((8, 128, 1152), np.float32)
        wd2 = np.zeros((8, 128, 128), np.float32)
        wdbt = np.zeros((128, 12), np.float32)
        wdbt2 = np.zeros((128, 1), np.float32)
        ow = np.zeros((8, 128, 1024), np.float32)
        ow2 = np.zeros((128, 1024), np.float32)
        for i, h in enumerate(heads):
            ow2[i * 32 : (i + 1) * 32, :] = out_w[h * 288 + 256 : h * 288 + 288, :]
            wd2[:, :, i * 32 : (i + 1) * 32] = (
                Wd_w[h][:, 256:288] / 8.0
            ).reshape(8, 128, 32)
            wdbt2[i * 32 : (i + 1) * 32, 0] = Wd_b[h][256:288] / 8.0
            wd[:, :, i * 288 : (i + 1) * 288] = (Wd_w[h] / 8.0).reshape(8, 128, 288)
            for lt, lsz in LT:
                wdbt[:lsz, i * 3 + lt] = Wd_b[h][lt * 128 : lt * 128 + lsz] / 8.0
                if lt < 2:
                    ow[i * 2 + lt, :lsz, :] = out_w[
                        h * 288 + lt * 128 : h * 288 + lt * 128 + lsz, :
                    ]
        grp_maps.append(
            {
                "wd": wd.astype(bf16),
                "wd2": wd2.astype(bf16),
                "wdbt": wdbt,
                "wdbt2": wdbt2,
                "ow": ow.astype(bf16),
                "ow2": ow2.astype(bf16),
            }
        )

    in_maps = []
    for c in range(NCORES):
        b, g = divmod(c, CPB)
        m = {
            "xT": xT[b],
            "lw": lw,
            "lbt": lbt,
            "masks": masks,
            "id128": id128,
        }
        m.update(grp_maps[g])
        in_maps.append(m)
    return in_maps


def _combine(results, out_b):
    out = np.zeros((B, T, C), np.float64)
    for c in range(NCORES):
        out[c // CPB] += results[c]["out_p"].astype(np.float64)
    out += np.asarray(out_b, dtype=np.float64)[None, None, :]
    return out.astype(np.float32)


def kernel(x, latent_w, latent_b, Wd_w, Wd_b, out_w, out_b, **kw):
    from concourse import bass_utils

    nc = _get_nc()
    in_maps = _prep_inputs(x, latent_w, latent_b, Wd_w, Wd_b, out_w)
    res = bass_utils.run_bass_kernel_spmd(nc, in_maps, core_ids=list(range(NCORES)))
    return _combine(res.results, out_b)
